# revision 7
# baseline (speedup 1.0000x reference)
"""ARERec forward kernel for 8 TRN2 NeuronCores.

Data-parallel over batch: each core processes B/8 = 64 batch rows end-to-end
(embedding gathers, single-query multi-head attention, LCU region profiles,
rating classifier); tables and weights are replicated. The final [512, 5]
softmax output is concatenated on the host from the 8 per-core [64, 5] shards.

Engine balance (per core, cost-model ns): the serial DMA bus moves ~52MB of
LCU rows (~150us floor); DVE holds the region-profile products; the fold
chains that reduce each product over e are split between DVE, DMA-accumulate
(CCE add) and Pool (gpsimd) so no single engine exceeds the DMA floor by
much. Both LCU tables are host-merged into one [ALLSEQ, 2*R*E] row so each
128-pair group needs one gather; item/neighbor rows use one mega-gather each.

Self-contained: shapes/sharding are hardcoded from the problem spec.
"""
import numpy as np
import ml_dtypes

import concourse.bacc as bacc
import concourse.bass as bass
import concourse.mybir as mybir
import concourse.tile as tile
from concourse.masks import make_identity
from concourse.bass_utils import run_bass_kernel_spmd

NCORES = 8
B, S, E, H, R = 512, 50, 128, 8, 32
D = E // H
USERS, ITEMS, ALLSEQ, NCLASS = 50000, 20000, 20000, 5
BC = B // NCORES            # 64 batch rows per core
J = BC * S                  # 3200 (b, s) pairs per core
G = J // 128                # 25 gather groups of 128 pairs
KROW = 2 * R * E            # 8192 combined-table row (kiu | kui)
# b-aligned chunks (multiples of S); two small priming chunks let the
# attention->profile software pipeline fill faster
CH400 = [(0, 200), (200, 200)] + [(c * 400, 400) for c in range(1, J // 400)]
# K-gather units: pairs of groups share one 2-column indirect gather
UNITS = [(u * 2, 2) for u in range(G // 2)] + [(G - 1, 1)]
# reduce mode per unit: 'a' all-DVE chain, 'b' fold1-DMA + DVE rest,
# 'c' fold1-DMA + Pool rest, 'd' all-Pool chain
MODES = ['b', 'd', 'a', 'd', 'b', 'd', 'a', 'd', 'b', 'd', 'a', 'b', 'a']
NB_CHUNKS = [(0, 9), (9, 8), (17, 8)]   # neighbor mega-gather splits
KPREFETCH = 2               # units of K gather issued ahead of compute

F32 = mybir.dt.float32
BF16 = mybir.dt.bfloat16
I32 = mybir.dt.int32
AF = mybir.ActivationFunctionType
ALU = mybir.AluOpType


def _ap(ap, dims):
    """Rebuild an AP with explicit [step, count] free dims (partition dim kept)."""
    return bass.AP(tensor=ap.tensor, offset=ap.offset, ap=[ap.ap[0]] + dims)


def _off(ap, extra_offset, dims):
    return bass.AP(tensor=ap.tensor, offset=ap.offset + extra_offset,
                   ap=[ap.ap[0]] + dims)


def build_nc(repeat=1):
    nc = bacc.Bacc(None, target_bir_lowering=False)

    p_user = nc.declare_dram_parameter("user_i", [BC, 1], I32, isOutput=False)
    p_nbg = nc.declare_dram_parameter("nbg_i", [128, G], I32, isOutput=False)
    p_seq = nc.declare_dram_parameter("seq_i", [128, G], I32, isOutput=False)
    p_itg = nc.declare_dram_parameter("itg_i", [128, G], I32, isOutput=False)
    p_negm = nc.declare_dram_parameter("negmask", [1, J], BF16, isOutput=False)
    p_uemb = nc.declare_dram_parameter("uemb", [USERS, E], F32, isOutput=False)
    p_item = nc.declare_dram_parameter("item_bf", [ITEMS, E], BF16, isOutput=False)
    p_kcat = nc.declare_dram_parameter("lcu_cat", [ALLSEQ, KROW], BF16, isOutput=False)
    p_wq = nc.declare_dram_parameter("wq", [E, E], F32, isOutput=False)
    p_wk = nc.declare_dram_parameter("wk", [E, E], BF16, isOutput=False)
    p_wv = nc.declare_dram_parameter("wv", [E, E], BF16, isOutput=False)
    p_wo = nc.declare_dram_parameter("wo", [E, E], F32, isOutput=False)
    p_bias = nc.declare_dram_parameter("biases", [E, 4], F32, isOutput=False)
    p_sel_eh = nc.declare_dram_parameter("sel_eh", [E, H], BF16, isOutput=False)
    p_sel_he = nc.declare_dram_parameter("sel_he", [H, E], BF16, isOutput=False)
    p_fcwb = nc.declare_dram_parameter("fc_wb", [R + 1, NCLASS], F32, isOutput=False)
    p_out = nc.declare_dram_parameter("out", [BC, NCLASS], F32, isOutput=True)
    p_ur = nc.declare_dram_parameter("ur_dbg", [R, BC], F32, isOutput=True)

    with tile.TileContext(nc) as tc:
        with (
            tc.tile_pool(name="const", bufs=1) as cpool,
            tc.tile_pool(name="big", bufs=1) as bpool,
            tc.tile_pool(name="work", bufs=3) as wpool,
            tc.tile_pool(name="kwork", bufs=3) as kpool,
            tc.tile_pool(name="ps_big", bufs=4, space="PSUM") as pp_big,
            tc.tile_pool(name="ps_lg", bufs=2, space="PSUM") as pp_lg,
            tc.tile_pool(name="ps_rt", bufs=1, space="PSUM") as pp_rt,
            tc.tile_pool(name="ps_l2", bufs=1, space="PSUM") as pp_l2,
        ):
            # ---------- constants (loaded once, outside the repeat loop) ----------
            t_ident = cpool.tile([128, 128], F32)
            make_identity(nc, t_ident[:])
            t_user = cpool.tile([BC, 1], I32)
            nc.sync.dma_start(out=t_user[:], in_=p_user[:])
            t_nbg = cpool.tile([128, G], I32)
            nc.sync.dma_start(out=t_nbg[:], in_=p_nbg[:])
            t_seq = cpool.tile([128, G], I32)
            nc.sync.dma_start(out=t_seq[:], in_=p_seq[:])
            t_itg = cpool.tile([128, G], I32)
            nc.sync.dma_start(out=t_itg[:], in_=p_itg[:])
            t_negm = cpool.tile([1, J], BF16)
            nc.sync.dma_start(out=t_negm[:], in_=p_negm[:])
            t_wq = cpool.tile([E, E], F32)
            nc.sync.dma_start(out=t_wq[:], in_=p_wq[:])
            t_wk = cpool.tile([E, E], BF16)
            nc.sync.dma_start(out=t_wk[:], in_=p_wk[:])
            t_wv = cpool.tile([E, E], BF16)
            nc.sync.dma_start(out=t_wv[:], in_=p_wv[:])
            t_wo = cpool.tile([E, E], F32)
            nc.sync.dma_start(out=t_wo[:], in_=p_wo[:])
            t_bias = cpool.tile([E, 4], F32)
            nc.sync.dma_start(out=t_bias[:], in_=p_bias[:])
            t_sel_eh = cpool.tile([E, H], BF16)
            nc.sync.dma_start(out=t_sel_eh[:], in_=p_sel_eh[:])
            t_sel_he = cpool.tile([H, E], BF16)
            nc.sync.dma_start(out=t_sel_he[:], in_=p_sel_he[:])
            t_fcwb = cpool.tile([R + 1, NCLASS], F32)
            nc.sync.dma_start(out=t_fcwb[:], in_=p_fcwb[:])
            t_ones18 = cpool.tile([1, H], BF16)
            nc.vector.memset(t_ones18[:], 1.0)
            # per-pair padding mask (1.0 where neighbor > 0), [128, G]
            t_wcol = cpool.tile([128, G], F32)
            nc.vector.tensor_scalar(out=t_wcol[:], in0=t_nbg[:], scalar1=0,
                                    scalar2=None, op0=ALU.is_gt)

            def body():
                # user rows -> [BC, E] -> transpose -> uT [E, BC]
                t_u = wpool.tile([BC, E], F32, tag="gath_u")
                nc.gpsimd.indirect_dma_start(
                    out=t_u[:], out_offset=None, in_=p_uemb[:],
                    in_offset=bass.IndirectOffsetOnAxis(ap=t_user[:, :1], axis=0))
                ps_uT = pp_big.tile([E, BC], F32, tag="big")
                nc.tensor.transpose(out=ps_uT[:], in_=t_u[:], identity=t_ident[:BC, :BC])
                t_uT = bpool.tile([E, BC], F32)
                nc.scalar.copy(out=t_uT[:], in_=ps_uT[:])

                # qT = (wq.T @ uT + bq) * (1/sqrt(D))  -- scale folded in here
                ps_q = pp_big.tile([E, BC], F32, tag="big")
                nc.tensor.matmul(out=ps_q[:], lhsT=t_wq[:], rhs=t_uT[:])
                t_qT = bpool.tile([E, BC], F32)
                nc.scalar.activation(out=t_qT[:], in_=ps_q[:], func=AF.Identity,
                                     bias=t_bias[:, 0:1], scale=1.0 / np.sqrt(D))

                # neighbor rows, mega-gathered in 3 chunks
                t_nball = bpool.tile([128, J], F32)
                t_nbT = bpool.tile([E, J], BF16)
                t_kT = bpool.tile([E, J], BF16)
                t_vT = bpool.tile([E, J], F32)
                t_att = bpool.tile([H, J], BF16)
                t_oT = bpool.tile([E, J], F32)
                t_ratT = bpool.tile([R, J], BF16)
                # multiplier tile: left half ctxo*w (pair-major), right half item
                t_mul = bpool.tile([128, 2 * J], BF16)
                nc.gpsimd.indirect_dma_start(
                    out=t_mul[:, J:2 * J], out_offset=None, in_=p_item[:],
                    in_offset=bass.IndirectOffsetOnAxis(ap=t_itg[:, 0:G], axis=0))

                def emit_nb_chunk(c0, cg):
                    nc.gpsimd.indirect_dma_start(
                        out=t_nball[:, c0 * 128:(c0 + cg) * 128], out_offset=None,
                        in_=p_uemb[:],
                        in_offset=bass.IndirectOffsetOnAxis(ap=t_nbg[:, c0:c0 + cg],
                                                            axis=0))

                def emit_nb_group(g):
                    # neighbor rows for group g, transposed into nbT slice
                    ps_t = pp_big.tile([128, 128], F32, tag="big")
                    nc.tensor.transpose(out=ps_t[:],
                                        in_=t_nball[:, g * 128:(g + 1) * 128],
                                        identity=t_ident[:])
                    nc.scalar.copy(out=t_nbT[:, g * 128:(g + 1) * 128], in_=ps_t[:])

                def emit_attn_a(ci, c0, cn):
                    sl = slice(c0, c0 + cn)
                    nb = cn // S
                    b0 = c0 // S
                    # k/v projections for this chunk
                    ps_k = pp_big.tile([E, 400], F32, tag="big")
                    nc.tensor.matmul(out=ps_k[:, :cn], lhsT=t_wk[:],
                                     rhs=t_nbT[:, sl])
                    nc.scalar.activation(out=t_kT[:, sl], in_=ps_k[:, :cn],
                                         func=AF.Identity, bias=t_bias[:, 1:2], scale=1.0)
                    ps_v = pp_big.tile([E, 400], F32, tag="big")
                    nc.tensor.matmul(out=ps_v[:, :cn], lhsT=t_wv[:],
                                     rhs=t_nbT[:, sl])
                    nc.scalar.activation(out=t_vT[:, sl], in_=ps_v[:, :cn],
                                         func=AF.Identity, bias=t_bias[:, 2:3], scale=1.0)
                    # prod_qk = kT * q_b (per-b Act pass; 1/sqrt(D) already in qT)
                    for bi in range(nb):
                        bsl = slice(c0 + bi * S, c0 + (bi + 1) * S)
                        nc.scalar.activation(out=t_kT[:, bsl], in_=t_kT[:, bsl],
                                             func=AF.Identity,
                                             scale=t_qT[:, b0 + bi:b0 + bi + 1])
                    # logits = per-head sums + negmask; attn = exp(logits)
                    # (masked entries underflow to exactly 0, matching the
                    # reference softmax); normalized by the row sum below.
                    ps_lg = pp_lg.tile([H, 400], F32, tag="lg")
                    nc.tensor.matmul(out=ps_lg[:, :cn], lhsT=t_sel_eh[:], rhs=t_kT[:, sl],
                                     start=True, stop=False)
                    nc.tensor.matmul(out=ps_lg[:, :cn], lhsT=t_ones18[:], rhs=t_negm[:, sl],
                                     start=False, stop=True)
                    t_ssc = wpool.tile([H, 8], F32, tag="ssc")
                    ss_tiles[ci] = t_ssc
                    for bi in range(cn // S):
                        nc.scalar.activation(out=t_att[:, c0 + bi * S:c0 + (bi + 1) * S],
                                             in_=ps_lg[:, bi * S:(bi + 1) * S], func=AF.Exp,
                                             accum_out=t_ssc[:, bi:bi + 1])

                def emit_attn_b(ci, c0, cn):
                    sl = slice(c0, c0 + cn)
                    nb = cn // S
                    t_ssc = ss_tiles.pop(ci)
                    t_rs = wpool.tile([H, 8], F32, tag="sm2")
                    nc.vector.reciprocal(out=t_rs[:, :nb], in_=t_ssc[:, :nb])
                    for bi in range(nb):
                        bsl = slice(c0 + bi * S, c0 + (bi + 1) * S)
                        nc.scalar.activation(out=t_att[:, bsl], in_=t_att[:, bsl],
                                             func=AF.Identity,
                                             scale=t_rs[:, bi:bi + 1])
                    # ctxT = attn_bcast * vT (in place over vT); ctxo = wo.T@ctx + bo
                    ps_ab = pp_big.tile([E, 400], F32, tag="big")
                    nc.tensor.matmul(out=ps_ab[:, :cn], lhsT=t_sel_he[:], rhs=t_att[:, sl])
                    nc.vector.tensor_tensor(out=t_vT[:, sl], in0=t_vT[:, sl],
                                            in1=ps_ab[:, :cn], op=ALU.mult)
                    ps_o = pp_big.tile([E, 400], F32, tag="big")
                    nc.tensor.matmul(out=ps_o[:, :cn], lhsT=t_wo[:], rhs=t_vT[:, sl])
                    nc.scalar.activation(out=t_oT[:, sl], in_=ps_o[:, :cn],
                                         func=AF.Identity, bias=t_bias[:, 3:4], scale=1.0)

                k_tiles = {}
                ss_tiles = {}

                def emit_k_gather(u):
                    g0, ug = UNITS[u]
                    t_k = kpool.tile([128, ug * KROW], BF16, tag="k")
                    k_tiles[u] = t_k
                    nc.gpsimd.indirect_dma_start(
                        out=t_k[:], out_offset=None, in_=p_kcat[:],
                        in_offset=bass.IndirectOffsetOnAxis(ap=t_seq[:, g0:g0 + ug],
                                                            axis=0))

                def emit_mul_prep(g):
                    # ctxo (pair-major, scaled by the padding mask) -> mul left half
                    ps_tp = pp_big.tile([128, 128], F32, tag="big")
                    nc.tensor.transpose(out=ps_tp[:], in_=t_oT[:, g * 128:(g + 1) * 128],
                                        identity=t_ident[:])
                    nc.scalar.activation(out=t_mul[:, g * 128:(g + 1) * 128],
                                         in_=ps_tp[:], func=AF.Identity,
                                         scale=t_wcol[:, g:g + 1])

                def emit_unit_compute(u):
                    g0, ug = UNITS[u]
                    mode = MODES[u]
                    t_k = k_tiles.pop(u)
                    # product: K[u,t,m,r,e] *= mul[(t),m-permuted e] broadcast over r
                    # per group (mul halves: t=0 ctxo' at cols g*128.., t=1 item at
                    # J + g*128..)
                    for gi in range(ug):
                        g = g0 + gi
                        k_in = _off(t_k[:], gi * KROW,
                                    [[R * E, 2], [R * E // 2, 2], [E // 2, R],
                                     [1, E // 2]])
                        mul_in = _off(t_mul[:], g * 128,
                                      [[J, 2], [E // 2, 2], [0, R], [1, E // 2]])
                        nc.vector.tensor_tensor(out=k_in, in0=k_in, in1=mul_in,
                                                op=ALU.mult)
                    # reduce over e via halving folds; layout per group chunk is
                    # (t, m, r, e_low) so fold1 halves m, later folds halve e_low
                    km = lambda h: _ap(t_k[:], [[KROW, ug], [R * E, 2], [E // 2, R],
                                               [1, h]])
                    kmh = lambda h: _off(t_k[:], h,
                                         [[KROW, ug], [R * E, 2], [E // 2, R], [1, h]])
                    # fold1: m=0 half += m=1 half (contiguous 2048-elem chunks)
                    f1_out = _ap(t_k[:], [[R * E, 2 * ug], [1, R * E // 2]])
                    f1_in = _off(t_k[:], R * E // 2, [[R * E, 2 * ug], [1, R * E // 2]])
                    if mode in ('b', 'c'):
                        nc.gpsimd.dma_start(out=f1_out, in_=f1_in, accum_op=ALU.add)
                    elif mode == 'd':
                        nc.gpsimd.tensor_tensor(out=f1_out, in0=f1_out, in1=f1_in,
                                                op=ALU.add)
                    else:
                        nc.vector.tensor_tensor(out=f1_out, in0=f1_out, in1=f1_in,
                                                op=ALU.add)
                    eng = nc.gpsimd if mode in ('c', 'd') else nc.vector
                    w = E // 2
                    while w > 1:
                        h = w // 2
                        eng.tensor_tensor(out=km(h), in0=km(h), in1=kmh(h), op=ALU.add)
                        w = h
                    # rating = nprof' * iprof (w already folded into ctxo side)
                    t_rat = wpool.tile([128, ug * R], F32, tag="rat")
                    rat_n = _ap(t_k[:], [[KROW, ug], [E // 2, R]])
                    rat_i = _off(t_k[:], R * E, [[KROW, ug], [E // 2, R]])
                    nc.vector.tensor_tensor(out=t_rat[:], in0=rat_n, in1=rat_i,
                                            op=ALU.mult)
                    # transpose [128, ug*R] -> [ug*R, 128] -> ratT rows
                    ps_rt = pp_rt.tile([2 * R, 128], F32, tag="rt")
                    nc.tensor.transpose(out=ps_rt[:ug * R, :], in_=t_rat[:],
                                        identity=t_ident[:])
                    for gi in range(ug):
                        g = g0 + gi
                        nc.scalar.copy(out=t_ratT[:, g * 128:(g + 1) * 128],
                                       in_=ps_rt[gi * R:(gi + 1) * R, :])

                # software-pipelined emission: neighbor transposes -> attention
                # stage A -> K-unit computes -> attention stage B -> mul preps
                nbc = 0
                g_nb = 0
                g_kg = 0
                g_prep = 0
                u_comp = 0
                for ci, (c0, cn) in enumerate(CH400):
                    hi = c0 + cn
                    while nbc < len(NB_CHUNKS) and NB_CHUNKS[nbc][0] * 128 < hi:
                        emit_nb_chunk(*NB_CHUNKS[nbc])
                        nbc += 1
                    while g_nb * 128 < hi:
                        emit_nb_group(g_nb)
                        g_nb += 1
                    while g_kg < len(UNITS) and UNITS[g_kg][0] * 128 <= hi + KPREFETCH * 256:
                        emit_k_gather(g_kg)
                        g_kg += 1
                    emit_attn_a(ci, c0, cn)
                    while u_comp < len(UNITS):
                        g0, ug = UNITS[u_comp]
                        if (g0 + ug) > g_prep - 1:
                            break
                        emit_unit_compute(u_comp)
                        u_comp += 1
                    emit_attn_b(ci, c0, cn)
                    while (g_prep + 1) * 128 <= hi:
                        emit_mul_prep(g_prep)
                        g_prep += 1
                while g_kg < len(UNITS):
                    emit_k_gather(g_kg)
                    g_kg += 1
                while g_prep < G:
                    emit_mul_prep(g_prep)
                    g_prep += 1
                while u_comp < len(UNITS):
                    emit_unit_compute(u_comp)
                    u_comp += 1

                # user rating vector: max over s
                t_urp = wpool.tile([R + 1, BC], F32, tag="urp")
                nc.vector.tensor_reduce(out=t_urp[:R, :],
                                        in_=t_ratT[:].rearrange("r (b s) -> r b s", s=S),
                                        axis=mybir.AxisListType.X, op=ALU.max)
                nc.vector.memset(t_urp[R:R + 1, :], 1.0)
                nc.sync.dma_start(out=p_ur[:], in_=t_urp[:R, :])

                # classifier + softmax
                ps_l2 = pp_l2.tile([BC, NCLASS], F32, tag="l2")
                nc.tensor.matmul(out=ps_l2[:], lhsT=t_urp[:], rhs=t_fcwb[:])
                t_nm2 = wpool.tile([BC, 1], F32, tag="fin")
                nc.vector.tensor_reduce(out=t_nm2[:], in_=ps_l2[:],
                                        axis=mybir.AxisListType.X,
                                        op=ALU.max, negate=True)
                t_e2 = wpool.tile([BC, NCLASS], F32, tag="fin2")
                t_s2 = wpool.tile([BC, 1], F32, tag="fin3")
                nc.scalar.activation(out=t_e2[:], in_=ps_l2[:], func=AF.Exp,
                                     bias=t_nm2[:, :1], scale=1.0,
                                     accum_out=t_s2[:, :1])
                t_r2 = wpool.tile([BC, 1], F32, tag="fin4")
                nc.vector.reciprocal(out=t_r2[:], in_=t_s2[:])
                t_o = wpool.tile([BC, NCLASS], F32, tag="fin5")
                nc.vector.tensor_scalar(out=t_o[:], in0=t_e2[:], scalar1=t_r2[:, :1],
                                        scalar2=None, op0=ALU.mult)
                nc.sync.dma_start(out=p_out[:], in_=t_o[:])

            if repeat == 1:
                body()
            else:
                with tc.For_i(0, repeat, 1):
                    body()

    nc.finalize()
    return nc


def prep_in_maps(inputs):
    user = np.asarray(inputs["user"]).astype(np.int32).reshape(B)
    item = np.asarray(inputs["item"]).astype(np.int32).reshape(B)
    neighbor = np.asarray(inputs["neighbor"]).astype(np.int32).reshape(B, S)
    seq = np.asarray(inputs["seq"]).astype(np.int32).reshape(B, S)

    f32 = lambda x: np.ascontiguousarray(np.asarray(x, dtype=np.float32))
    bf16 = lambda x: np.ascontiguousarray(
        np.asarray(x, dtype=np.float32).astype(ml_dtypes.bfloat16))

    uemb = f32(inputs["user_emb_table"])
    item_bf = bf16(inputs["item_emb_table"])
    # both LCU tables in one row: (t, e_msb, r, e_low) so the first fold's
    # halves are contiguous 2048-elem chunks per table
    perm = lambda t: t.reshape(ALLSEQ, R, 2, E // 2).transpose(0, 2, 1, 3).reshape(
        ALLSEQ, R * E)
    lcu_cat = np.ascontiguousarray(np.concatenate(
        [perm(bf16(inputs["item_user_LCU"])),
         perm(bf16(inputs["user_item_LCU"]))], axis=1))
    biases = np.ascontiguousarray(np.stack(
        [f32(inputs["bq"]), f32(inputs["bk"]),
         f32(inputs["bv"]), f32(inputs["bo"])], axis=1))
    sel_eh = np.zeros((E, H), np.float32)
    sel_eh[np.arange(E), np.arange(E) // D] = 1.0
    fc_wb = np.ascontiguousarray(np.concatenate(
        [f32(inputs["fc_w"]), f32(inputs["fc_b"]).reshape(1, NCLASS)], axis=0))

    shared = {
        "uemb": uemb, "item_bf": item_bf, "lcu_cat": lcu_cat,
        "wq": f32(inputs["wq"]), "wk": bf16(inputs["wk"]),
        "wv": bf16(inputs["wv"]), "wo": f32(inputs["wo"]),
        "biases": biases, "sel_eh": np.ascontiguousarray(sel_eh.astype(ml_dtypes.bfloat16)),
        "sel_he": np.ascontiguousarray(sel_eh.T.astype(ml_dtypes.bfloat16)), "fc_wb": fc_wb,
    }
    in_maps = []
    for c in range(NCORES):
        bsl = slice(c * BC, (c + 1) * BC)
        nb = neighbor[bsl].reshape(J)
        sq = seq[bsl].reshape(J)
        itx = np.repeat(item[bsl], S)
        col = lambda x: np.ascontiguousarray(x.reshape(G, 128).T.astype(np.int32))
        in_maps.append({
            **shared,
            "user_i": np.ascontiguousarray(user[bsl].reshape(BC, 1)),
            "nbg_i": col(nb), "seq_i": col(sq), "itg_i": col(itx),
            "negmask": np.ascontiguousarray(
                (-1e9 * (nb <= 0)).astype(ml_dtypes.bfloat16).reshape(1, J)),
        })
    return in_maps


_NC_CACHE = {}


def kernel(**inputs):
    if "nc" not in _NC_CACHE:
        _NC_CACHE["nc"] = build_nc(repeat=1)
    nc = _NC_CACHE["nc"]
    in_maps = prep_in_maps(inputs)
    res = run_bass_kernel_spmd(nc, in_maps, core_ids=list(range(NCORES)))
    return np.concatenate([res.results[c]["out"] for c in range(NCORES)], axis=0)


# revision 11
# speedup vs baseline: 1.1027x; 1.1027x over previous
"""ARERec forward kernel for 8 TRN2 NeuronCores.

Data-parallel over batch: each core processes B/8 = 64 batch rows end-to-end
(embedding gathers, single-query multi-head attention, LCU region profiles,
rating classifier); tables and weights are replicated. The final [512, 5]
softmax output is concatenated on the host from the 8 per-core [64, 5] shards.

Engine balance (per core, cost-model ns): the serial DMA bus moves ~52MB of
LCU rows (~150us floor); DVE holds the region-profile products; the fold
chains that reduce each product over e are split between DVE, DMA-accumulate
(CCE add) and Pool (gpsimd) so no single engine exceeds the DMA floor by
much. Both LCU tables are host-merged into one [ALLSEQ, 2*R*E] row so each
128-pair group needs one gather; item/neighbor rows use one mega-gather each.

Self-contained: shapes/sharding are hardcoded from the problem spec.
"""
import numpy as np
import ml_dtypes

import concourse.bacc as bacc
import concourse.bass as bass
import concourse.mybir as mybir
import concourse.tile as tile
from concourse.masks import make_identity
from concourse.bass_utils import run_bass_kernel_spmd

NCORES = 8
B, S, E, H, R = 512, 50, 128, 8, 32
D = E // H
USERS, ITEMS, ALLSEQ, NCLASS = 50000, 20000, 20000, 5
BC = B // NCORES            # 64 batch rows per core
J = BC * S                  # 3200 (b, s) pairs per core
G = J // 128                # 25 gather groups of 128 pairs
KROW = 2 * R * E            # 8192 combined-table row (kiu | kui)
# b-aligned chunks (multiples of S); two small priming chunks let the
# attention->profile software pipeline fill faster
CH400 = [(0, 200), (200, 200)] + [(c * 400, 400) for c in range(1, J // 400)]
# K-gather units: pairs of groups share one 2-column indirect gather
UNITS = [(u * 2, 2) for u in range(G // 2)] + [(G - 1, 1)]
# reduce mode per group: 'a' all-DVE chain, 'b' fold1-DMA + DVE rest,
# 'c' fold1-DMA + Pool rest, 'e' fold1-DVE + Pool rest
MODES = ['e', 'b', 'e', 'a', 'e', 'b', 'e', 'e', 'b', 'a', 'e', 'b', 'e',
         'e', 'b', 'a', 'e', 'b', 'e', 'e', 'b', 'a', 'e', 'c', 'c']
NB_CHUNKS = [(0, 9), (9, 8), (17, 8)]   # neighbor mega-gather splits
KPREFETCH = 3               # groups of K gather issued ahead of compute

F32 = mybir.dt.float32
BF16 = mybir.dt.bfloat16
I32 = mybir.dt.int32
AF = mybir.ActivationFunctionType
ALU = mybir.AluOpType


def _ap(ap, dims):
    """Rebuild an AP with explicit [step, count] free dims (partition dim kept)."""
    return bass.AP(tensor=ap.tensor, offset=ap.offset, ap=[ap.ap[0]] + dims)


def _off(ap, extra_offset, dims):
    return bass.AP(tensor=ap.tensor, offset=ap.offset + extra_offset,
                   ap=[ap.ap[0]] + dims)


def build_nc(repeat=1):
    nc = bacc.Bacc(None, target_bir_lowering=False)

    p_user = nc.declare_dram_parameter("user_i", [BC, 1], I32, isOutput=False)
    p_nbg = nc.declare_dram_parameter("nbg_i", [128, G], I32, isOutput=False)
    p_seq = nc.declare_dram_parameter("seq_i", [128, G], I32, isOutput=False)
    p_itg = nc.declare_dram_parameter("itg_i", [128, G], I32, isOutput=False)
    p_negm = nc.declare_dram_parameter("negmask", [1, J], BF16, isOutput=False)
    p_uemb = nc.declare_dram_parameter("uemb", [USERS, E], F32, isOutput=False)
    p_item = nc.declare_dram_parameter("item_bf", [ITEMS, E], BF16, isOutput=False)
    p_kcat = nc.declare_dram_parameter("lcu_cat", [ALLSEQ, KROW], BF16, isOutput=False)
    p_wq = nc.declare_dram_parameter("wq", [E, E], F32, isOutput=False)
    p_wk = nc.declare_dram_parameter("wk", [E, E], BF16, isOutput=False)
    p_wv = nc.declare_dram_parameter("wv", [E, E], BF16, isOutput=False)
    p_wo = nc.declare_dram_parameter("wo", [E, E], F32, isOutput=False)
    p_bias = nc.declare_dram_parameter("biases", [E, 4], F32, isOutput=False)
    p_sel_eh = nc.declare_dram_parameter("sel_eh", [E, H], BF16, isOutput=False)
    p_sel_he = nc.declare_dram_parameter("sel_he", [H, E], BF16, isOutput=False)
    p_fcwb = nc.declare_dram_parameter("fc_wb", [R + 1, NCLASS], F32, isOutput=False)
    p_out = nc.declare_dram_parameter("out", [BC, NCLASS], F32, isOutput=True)
    p_ur = nc.declare_dram_parameter("ur_dbg", [R, BC], F32, isOutput=True)

    with tile.TileContext(nc) as tc:
        with (
            tc.tile_pool(name="const", bufs=1) as cpool,
            tc.tile_pool(name="big", bufs=1) as bpool,
            tc.tile_pool(name="work", bufs=3) as wpool,
            tc.tile_pool(name="kwork", bufs=3) as kpool,
            tc.tile_pool(name="ps_big", bufs=4, space="PSUM") as pp_big,
            tc.tile_pool(name="ps_lg", bufs=2, space="PSUM") as pp_lg,
            tc.tile_pool(name="ps_rt", bufs=1, space="PSUM") as pp_rt,
            tc.tile_pool(name="ps_l2", bufs=1, space="PSUM") as pp_l2,
        ):
            # ---------- constants (loaded once, outside the repeat loop) ----------
            t_ident = cpool.tile([128, 128], F32)
            make_identity(nc, t_ident[:])
            t_user = cpool.tile([BC, 1], I32)
            nc.sync.dma_start(out=t_user[:], in_=p_user[:])
            t_nbg = cpool.tile([128, G], I32)
            nc.sync.dma_start(out=t_nbg[:], in_=p_nbg[:])
            t_seq = cpool.tile([128, G], I32)
            nc.sync.dma_start(out=t_seq[:], in_=p_seq[:])
            t_itg = cpool.tile([128, G], I32)
            nc.sync.dma_start(out=t_itg[:], in_=p_itg[:])
            t_negm = cpool.tile([1, J], BF16)
            nc.sync.dma_start(out=t_negm[:], in_=p_negm[:])
            t_wq = cpool.tile([E, E], F32)
            nc.sync.dma_start(out=t_wq[:], in_=p_wq[:])
            t_wk = cpool.tile([E, E], BF16)
            nc.sync.dma_start(out=t_wk[:], in_=p_wk[:])
            t_wv = cpool.tile([E, E], BF16)
            nc.sync.dma_start(out=t_wv[:], in_=p_wv[:])
            t_wo = cpool.tile([E, E], F32)
            nc.sync.dma_start(out=t_wo[:], in_=p_wo[:])
            t_bias = cpool.tile([E, 4], F32)
            nc.sync.dma_start(out=t_bias[:], in_=p_bias[:])
            t_sel_eh = cpool.tile([E, H], BF16)
            nc.sync.dma_start(out=t_sel_eh[:], in_=p_sel_eh[:])
            t_sel_he = cpool.tile([H, E], BF16)
            nc.sync.dma_start(out=t_sel_he[:], in_=p_sel_he[:])
            t_fcwb = cpool.tile([R + 1, NCLASS], F32)
            nc.sync.dma_start(out=t_fcwb[:], in_=p_fcwb[:])
            t_ones18 = cpool.tile([1, H], BF16)
            nc.vector.memset(t_ones18[:], 1.0)
            # per-pair padding mask (1.0 where neighbor > 0), [128, G]
            t_wcol = cpool.tile([128, G], F32)
            nc.vector.tensor_scalar(out=t_wcol[:], in0=t_nbg[:], scalar1=0,
                                    scalar2=None, op0=ALU.is_gt)

            def body():
                # user rows -> [BC, E] -> transpose -> uT [E, BC]
                t_u = wpool.tile([BC, E], F32, tag="gath_u")
                nc.gpsimd.indirect_dma_start(
                    out=t_u[:], out_offset=None, in_=p_uemb[:],
                    in_offset=bass.IndirectOffsetOnAxis(ap=t_user[:, :1], axis=0))
                ps_uT = pp_big.tile([E, BC], F32, tag="big")
                nc.tensor.transpose(out=ps_uT[:], in_=t_u[:], identity=t_ident[:BC, :BC])
                t_uT = bpool.tile([E, BC], F32)
                nc.scalar.copy(out=t_uT[:], in_=ps_uT[:])

                # qT = (wq.T @ uT + bq) * (1/sqrt(D))  -- scale folded in here
                ps_q = pp_big.tile([E, BC], F32, tag="big")
                nc.tensor.matmul(out=ps_q[:], lhsT=t_wq[:], rhs=t_uT[:])
                t_qT = bpool.tile([E, BC], F32)
                nc.scalar.activation(out=t_qT[:], in_=ps_q[:], func=AF.Identity,
                                     bias=t_bias[:, 0:1], scale=1.0 / np.sqrt(D))

                # neighbor rows, mega-gathered in 3 chunks
                t_nball = bpool.tile([128, J], F32)
                t_nbT = bpool.tile([E, J], BF16)
                t_kT = bpool.tile([E, J], BF16)
                t_vT = bpool.tile([E, J], F32)
                t_att = bpool.tile([H, J], BF16)
                t_oT = bpool.tile([E, J], F32)
                t_ratT = bpool.tile([R, J], BF16)
                # multiplier tile: left half ctxo*w (pair-major), right half item
                t_mul = bpool.tile([128, 2 * J], BF16)
                with tc.high_priority():
                    nc.gpsimd.indirect_dma_start(
                        out=t_mul[:, J:2 * J], out_offset=None, in_=p_item[:],
                        in_offset=bass.IndirectOffsetOnAxis(ap=t_itg[:, 0:G], axis=0))

                def emit_nb_chunk(c0, cg):
                    with tc.high_priority():
                        nc.gpsimd.indirect_dma_start(
                            out=t_nball[:, c0 * 128:(c0 + cg) * 128], out_offset=None,
                            in_=p_uemb[:],
                            in_offset=bass.IndirectOffsetOnAxis(ap=t_nbg[:, c0:c0 + cg],
                                                                axis=0))

                def emit_nb_group(g):
                    # neighbor rows for group g, transposed into nbT slice
                    ps_t = pp_big.tile([128, 128], F32, tag="big")
                    nc.tensor.transpose(out=ps_t[:],
                                        in_=t_nball[:, g * 128:(g + 1) * 128],
                                        identity=t_ident[:])
                    nc.scalar.copy(out=t_nbT[:, g * 128:(g + 1) * 128], in_=ps_t[:])

                def emit_attn_a(ci, c0, cn):
                    sl = slice(c0, c0 + cn)
                    nb = cn // S
                    b0 = c0 // S
                    # k/v projections for this chunk
                    ps_k = pp_big.tile([E, 400], F32, tag="big")
                    nc.tensor.matmul(out=ps_k[:, :cn], lhsT=t_wk[:],
                                     rhs=t_nbT[:, sl])
                    nc.scalar.activation(out=t_kT[:, sl], in_=ps_k[:, :cn],
                                         func=AF.Identity, bias=t_bias[:, 1:2], scale=1.0)
                    ps_v = pp_big.tile([E, 400], F32, tag="big")
                    nc.tensor.matmul(out=ps_v[:, :cn], lhsT=t_wv[:],
                                     rhs=t_nbT[:, sl])
                    nc.scalar.activation(out=t_vT[:, sl], in_=ps_v[:, :cn],
                                         func=AF.Identity, bias=t_bias[:, 2:3], scale=1.0)
                    # prod_qk = kT * q_b (per-b Act pass; 1/sqrt(D) already in qT)
                    for bi in range(nb):
                        bsl = slice(c0 + bi * S, c0 + (bi + 1) * S)
                        nc.scalar.activation(out=t_kT[:, bsl], in_=t_kT[:, bsl],
                                             func=AF.Identity,
                                             scale=t_qT[:, b0 + bi:b0 + bi + 1])
                    # logits = per-head sums + negmask; attn = exp(logits)
                    # (masked entries underflow to exactly 0, matching the
                    # reference softmax); normalized by the row sum below.
                    ps_lg = pp_lg.tile([H, 400], F32, tag="lg")
                    nc.tensor.matmul(out=ps_lg[:, :cn], lhsT=t_sel_eh[:], rhs=t_kT[:, sl],
                                     start=True, stop=False)
                    nc.tensor.matmul(out=ps_lg[:, :cn], lhsT=t_ones18[:], rhs=t_negm[:, sl],
                                     start=False, stop=True)
                    t_ssc = wpool.tile([H, 8], F32, tag="ssc")
                    ss_tiles[ci] = t_ssc
                    for bi in range(cn // S):
                        nc.scalar.activation(out=t_att[:, c0 + bi * S:c0 + (bi + 1) * S],
                                             in_=ps_lg[:, bi * S:(bi + 1) * S], func=AF.Exp,
                                             accum_out=t_ssc[:, bi:bi + 1])

                def emit_attn_b(ci, c0, cn):
                    sl = slice(c0, c0 + cn)
                    nb = cn // S
                    t_ssc = ss_tiles.pop(ci)
                    t_rs = wpool.tile([H, 8], F32, tag="sm2")
                    nc.vector.reciprocal(out=t_rs[:, :nb], in_=t_ssc[:, :nb])
                    for bi in range(nb):
                        bsl = slice(c0 + bi * S, c0 + (bi + 1) * S)
                        nc.scalar.activation(out=t_att[:, bsl], in_=t_att[:, bsl],
                                             func=AF.Identity,
                                             scale=t_rs[:, bi:bi + 1])
                    # ctxT = attn_bcast * vT (in place over vT); ctxo = wo.T@ctx + bo
                    ps_ab = pp_big.tile([E, 400], F32, tag="big")
                    nc.tensor.matmul(out=ps_ab[:, :cn], lhsT=t_sel_he[:], rhs=t_att[:, sl])
                    nc.vector.tensor_tensor(out=t_vT[:, sl], in0=t_vT[:, sl],
                                            in1=ps_ab[:, :cn], op=ALU.mult)
                    ps_o = pp_big.tile([E, 400], F32, tag="big")
                    nc.tensor.matmul(out=ps_o[:, :cn], lhsT=t_wo[:], rhs=t_vT[:, sl])
                    nc.scalar.activation(out=t_oT[:, sl], in_=ps_o[:, :cn],
                                         func=AF.Identity, bias=t_bias[:, 3:4], scale=1.0)

                k_tiles = {}
                ss_tiles = {}

                def emit_k_gather(u):
                    g0, ug = UNITS[u]
                    t_k = kpool.tile([128, ug * KROW], BF16, tag="k")
                    k_tiles[u] = t_k
                    with tc.high_priority():
                        nc.gpsimd.indirect_dma_start(
                            out=t_k[:], out_offset=None, in_=p_kcat[:],
                            in_offset=bass.IndirectOffsetOnAxis(ap=t_seq[:, g0:g0 + ug],
                                                                axis=0))

                def emit_mul_prep(g):
                    # ctxo (pair-major, scaled by the padding mask) -> mul left half
                    ps_tp = pp_big.tile([128, 128], F32, tag="big")
                    nc.tensor.transpose(out=ps_tp[:], in_=t_oT[:, g * 128:(g + 1) * 128],
                                        identity=t_ident[:])
                    nc.scalar.activation(out=t_mul[:, g * 128:(g + 1) * 128],
                                         in_=ps_tp[:], func=AF.Identity,
                                         scale=t_wcol[:, g:g + 1])

                def emit_unit_compute(u):
                    g0, ug = UNITS[u]
                    t_k = k_tiles.pop(u)
                    for gi in range(ug):
                        g = g0 + gi
                        mode = MODES[g]
                        off = gi * KROW
                        # product: K[t,m,r,e] *= mul[(t),m-permuted e] broadcast
                        # over r (mul halves: t=0 ctxo' at cols g*128.., t=1 item
                        # at J + g*128..)
                        k_in = _off(t_k[:], off,
                                    [[R * E, 2], [R * E // 2, 2], [E // 2, R],
                                     [1, E // 2]])
                        mul_in = _off(t_mul[:], g * 128,
                                      [[J, 2], [E // 2, 2], [0, R], [1, E // 2]])
                        nc.vector.tensor_tensor(out=k_in, in0=k_in, in1=mul_in,
                                                op=ALU.mult)
                        # reduce over e via halving folds; layout per group chunk
                        # is (t, m, r, e_low): fold1 halves m, later folds e_low
                        km = lambda h: _off(t_k[:], off,
                                            [[R * E, 2], [E // 2, R], [1, h]])
                        kmh = lambda h: _off(t_k[:], off + h,
                                             [[R * E, 2], [E // 2, R], [1, h]])
                        # fold1: m=0 half += m=1 half (contiguous 2048-elem chunks)
                        f1_out = _off(t_k[:], off, [[R * E, 2], [1, R * E // 2]])
                        f1_in = _off(t_k[:], off + R * E // 2,
                                     [[R * E, 2], [1, R * E // 2]])
                        if mode in ('b', 'c'):
                            nc.gpsimd.dma_start(out=f1_out, in_=f1_in,
                                                accum_op=ALU.add)
                        else:
                            nc.vector.tensor_tensor(out=f1_out, in0=f1_out,
                                                    in1=f1_in, op=ALU.add)
                        eng = nc.gpsimd if mode in ('c', 'e') else nc.vector
                        w = E // 2
                        while w > 1:
                            h = w // 2
                            eng.tensor_tensor(out=km(h), in0=km(h), in1=kmh(h),
                                              op=ALU.add)
                            w = h
                    # rating = nprof' * iprof (w already folded into ctxo side)
                    t_rat = wpool.tile([128, ug * R], F32, tag="rat")
                    rat_n = _ap(t_k[:], [[KROW, ug], [E // 2, R]])
                    rat_i = _off(t_k[:], R * E, [[KROW, ug], [E // 2, R]])
                    nc.vector.tensor_tensor(out=t_rat[:], in0=rat_n, in1=rat_i,
                                            op=ALU.mult)
                    # transpose [128, ug*R] -> [ug*R, 128] -> ratT rows
                    ps_rt = pp_rt.tile([2 * R, 128], F32, tag="rt")
                    nc.tensor.transpose(out=ps_rt[:ug * R, :], in_=t_rat[:],
                                        identity=t_ident[:])
                    for gi in range(ug):
                        g = g0 + gi
                        nc.scalar.copy(out=t_ratT[:, g * 128:(g + 1) * 128],
                                       in_=ps_rt[gi * R:(gi + 1) * R, :])

                # software-pipelined emission: neighbor transposes -> attention
                # stage A -> K-unit computes -> attention stage B -> mul preps
                nbc = 0
                g_nb = 0
                g_kg = 0
                g_prep = 0
                u_comp = 0
                for ci, (c0, cn) in enumerate(CH400):
                    hi = c0 + cn
                    while nbc < len(NB_CHUNKS) and NB_CHUNKS[nbc][0] * 128 < hi:
                        emit_nb_chunk(*NB_CHUNKS[nbc])
                        nbc += 1
                    while g_nb * 128 < hi:
                        emit_nb_group(g_nb)
                        g_nb += 1
                    while g_kg < len(UNITS) and UNITS[g_kg][0] * 128 <= hi + KPREFETCH * 256:
                        emit_k_gather(g_kg)
                        g_kg += 1
                    emit_attn_a(ci, c0, cn)
                    while u_comp < len(UNITS):
                        g0, ug = UNITS[u_comp]
                        if (g0 + ug) > g_prep - 1:
                            break
                        emit_unit_compute(u_comp)
                        u_comp += 1
                    emit_attn_b(ci, c0, cn)
                    while (g_prep + 1) * 128 <= hi:
                        emit_mul_prep(g_prep)
                        g_prep += 1
                while g_kg < len(UNITS):
                    emit_k_gather(g_kg)
                    g_kg += 1
                while g_prep < G:
                    emit_mul_prep(g_prep)
                    g_prep += 1
                while u_comp < len(UNITS):
                    emit_unit_compute(u_comp)
                    u_comp += 1

                # user rating vector: max over s
                t_urp = wpool.tile([R + 1, BC], F32, tag="urp")
                nc.vector.tensor_reduce(out=t_urp[:R, :],
                                        in_=t_ratT[:].rearrange("r (b s) -> r b s", s=S),
                                        axis=mybir.AxisListType.X, op=ALU.max)
                nc.vector.memset(t_urp[R:R + 1, :], 1.0)
                nc.sync.dma_start(out=p_ur[:], in_=t_urp[:R, :])

                # classifier + softmax
                ps_l2 = pp_l2.tile([BC, NCLASS], F32, tag="l2")
                nc.tensor.matmul(out=ps_l2[:], lhsT=t_urp[:], rhs=t_fcwb[:])
                t_nm2 = wpool.tile([BC, 1], F32, tag="fin")
                nc.vector.tensor_reduce(out=t_nm2[:], in_=ps_l2[:],
                                        axis=mybir.AxisListType.X,
                                        op=ALU.max, negate=True)
                t_e2 = wpool.tile([BC, NCLASS], F32, tag="fin2")
                t_s2 = wpool.tile([BC, 1], F32, tag="fin3")
                nc.scalar.activation(out=t_e2[:], in_=ps_l2[:], func=AF.Exp,
                                     bias=t_nm2[:, :1], scale=1.0,
                                     accum_out=t_s2[:, :1])
                t_r2 = wpool.tile([BC, 1], F32, tag="fin4")
                nc.vector.reciprocal(out=t_r2[:], in_=t_s2[:])
                t_o = wpool.tile([BC, NCLASS], F32, tag="fin5")
                nc.vector.tensor_scalar(out=t_o[:], in0=t_e2[:], scalar1=t_r2[:, :1],
                                        scalar2=None, op0=ALU.mult)
                nc.sync.dma_start(out=p_out[:], in_=t_o[:])

            if repeat == 1:
                body()
            else:
                with tc.For_i(0, repeat, 1):
                    body()

    nc.finalize()
    return nc


def prep_in_maps(inputs):
    user = np.asarray(inputs["user"]).astype(np.int32).reshape(B)
    item = np.asarray(inputs["item"]).astype(np.int32).reshape(B)
    neighbor = np.asarray(inputs["neighbor"]).astype(np.int32).reshape(B, S)
    seq = np.asarray(inputs["seq"]).astype(np.int32).reshape(B, S)

    f32 = lambda x: np.ascontiguousarray(np.asarray(x, dtype=np.float32))
    bf16 = lambda x: np.ascontiguousarray(
        np.asarray(x, dtype=np.float32).astype(ml_dtypes.bfloat16))

    uemb = f32(inputs["user_emb_table"])
    item_bf = bf16(inputs["item_emb_table"])
    # both LCU tables in one row: (t, e_msb, r, e_low) so the first fold's
    # halves are contiguous 2048-elem chunks per table
    perm = lambda t: t.reshape(ALLSEQ, R, 2, E // 2).transpose(0, 2, 1, 3).reshape(
        ALLSEQ, R * E)
    lcu_cat = np.ascontiguousarray(np.concatenate(
        [perm(bf16(inputs["item_user_LCU"])),
         perm(bf16(inputs["user_item_LCU"]))], axis=1))
    biases = np.ascontiguousarray(np.stack(
        [f32(inputs["bq"]), f32(inputs["bk"]),
         f32(inputs["bv"]), f32(inputs["bo"])], axis=1))
    sel_eh = np.zeros((E, H), np.float32)
    sel_eh[np.arange(E), np.arange(E) // D] = 1.0
    fc_wb = np.ascontiguousarray(np.concatenate(
        [f32(inputs["fc_w"]), f32(inputs["fc_b"]).reshape(1, NCLASS)], axis=0))

    shared = {
        "uemb": uemb, "item_bf": item_bf, "lcu_cat": lcu_cat,
        "wq": f32(inputs["wq"]), "wk": bf16(inputs["wk"]),
        "wv": bf16(inputs["wv"]), "wo": f32(inputs["wo"]),
        "biases": biases, "sel_eh": np.ascontiguousarray(sel_eh.astype(ml_dtypes.bfloat16)),
        "sel_he": np.ascontiguousarray(sel_eh.T.astype(ml_dtypes.bfloat16)), "fc_wb": fc_wb,
    }
    in_maps = []
    for c in range(NCORES):
        bsl = slice(c * BC, (c + 1) * BC)
        nb = neighbor[bsl].reshape(J)
        sq = seq[bsl].reshape(J)
        itx = np.repeat(item[bsl], S)
        col = lambda x: np.ascontiguousarray(x.reshape(G, 128).T.astype(np.int32))
        in_maps.append({
            **shared,
            "user_i": np.ascontiguousarray(user[bsl].reshape(BC, 1)),
            "nbg_i": col(nb), "seq_i": col(sq), "itg_i": col(itx),
            "negmask": np.ascontiguousarray(
                (-1e9 * (nb <= 0)).astype(ml_dtypes.bfloat16).reshape(1, J)),
        })
    return in_maps


_NC_CACHE = {}


def kernel(**inputs):
    if "nc" not in _NC_CACHE:
        _NC_CACHE["nc"] = build_nc(repeat=1)
    nc = _NC_CACHE["nc"]
    in_maps = prep_in_maps(inputs)
    res = run_bass_kernel_spmd(nc, in_maps, core_ids=list(range(NCORES)))
    return np.concatenate([res.results[c]["out"] for c in range(NCORES)], axis=0)


# revision 14
# speedup vs baseline: 1.1064x; 1.0034x over previous
"""ARERec forward kernel for 8 TRN2 NeuronCores.

Data-parallel over batch: each core processes B/8 = 64 batch rows end-to-end
(embedding gathers, single-query multi-head attention, LCU region profiles,
rating classifier); tables and weights are replicated. The final [512, 5]
softmax output is concatenated on the host from the 8 per-core [64, 5] shards.

Engine balance (per core, cost-model ns): the serial DMA bus moves ~52MB of
LCU rows (~150us floor); DVE holds the region-profile products; the fold
chains that reduce each product over e are split between DVE, DMA-accumulate
(CCE add) and Pool (gpsimd) so no single engine exceeds the DMA floor by
much. Both LCU tables are host-merged into one [ALLSEQ, 2*R*E] row so each
128-pair group needs one gather; item/neighbor rows use one mega-gather each.

Self-contained: shapes/sharding are hardcoded from the problem spec.
"""
import numpy as np
import ml_dtypes

import concourse.bacc as bacc
import concourse.bass as bass
import concourse.mybir as mybir
import concourse.tile as tile
from concourse.masks import make_identity
from concourse.bass_utils import run_bass_kernel_spmd

NCORES = 8
B, S, E, H, R = 512, 50, 128, 8, 32
D = E // H
USERS, ITEMS, ALLSEQ, NCLASS = 50000, 20000, 20000, 5
BC = B // NCORES            # 64 batch rows per core
J = BC * S                  # 3200 (b, s) pairs per core
G = J // 128                # 25 gather groups of 128 pairs
KROW = 2 * R * E            # 8192 combined-table row (kiu | kui)
# b-aligned chunks (multiples of S); two small priming chunks let the
# attention->profile software pipeline fill faster
CH400 = [(0, 200), (200, 200)] + [(c * 400, 400) for c in range(1, J // 400)]
# K-gather units: pairs of groups share one 2-column indirect gather
UNITS = [(u * 2, 2) for u in range(G // 2)] + [(G - 1, 1)]
# reduce mode per group: 'a' all-DVE chain, 'b' fold1-DMA + DVE rest,
# 'c' fold1-DMA + Pool rest, 'e' fold1-DVE + Pool rest
MODES = ['e', 'b', 'e', 'a', 'e', 'b', 'e', 'e', 'b', 'a', 'e', 'b', 'e',
         'e', 'b', 'a', 'e', 'b', 'e', 'e', 'b', 'a', 'e', 'c', 'c']
NB_CHUNKS = [(0, 9), (9, 8), (17, 8)]   # neighbor mega-gather splits
KPREFETCH = 3               # groups of K gather issued ahead of compute

F32 = mybir.dt.float32
BF16 = mybir.dt.bfloat16
I32 = mybir.dt.int32
AF = mybir.ActivationFunctionType
ALU = mybir.AluOpType


def _ap(ap, dims):
    """Rebuild an AP with explicit [step, count] free dims (partition dim kept)."""
    return bass.AP(tensor=ap.tensor, offset=ap.offset, ap=[ap.ap[0]] + dims)


def _off(ap, extra_offset, dims):
    return bass.AP(tensor=ap.tensor, offset=ap.offset + extra_offset,
                   ap=[ap.ap[0]] + dims)


def build_nc(repeat=1):
    nc = bacc.Bacc(None, target_bir_lowering=False)

    p_user = nc.declare_dram_parameter("user_i", [BC, 1], I32, isOutput=False)
    p_nbg = nc.declare_dram_parameter("nbg_i", [128, G], I32, isOutput=False)
    p_seq = nc.declare_dram_parameter("seq_i", [128, G], I32, isOutput=False)
    p_itg = nc.declare_dram_parameter("itg_i", [128, G], I32, isOutput=False)
    p_negm = nc.declare_dram_parameter("negmask", [1, J], BF16, isOutput=False)
    p_uemb = nc.declare_dram_parameter("uemb", [USERS, E], F32, isOutput=False)
    p_item = nc.declare_dram_parameter("item_bf", [ITEMS, E], BF16, isOutput=False)
    p_kcat = nc.declare_dram_parameter("lcu_cat", [ALLSEQ, KROW], BF16, isOutput=False)
    p_wq = nc.declare_dram_parameter("wq", [E, E], F32, isOutput=False)
    p_wk = nc.declare_dram_parameter("wk", [E, E], BF16, isOutput=False)
    p_wv = nc.declare_dram_parameter("wv", [E, E], BF16, isOutput=False)
    p_wo = nc.declare_dram_parameter("wo", [E, E], F32, isOutput=False)
    p_bias = nc.declare_dram_parameter("biases", [E, 4], F32, isOutput=False)
    p_sel_eh = nc.declare_dram_parameter("sel_eh", [E, H], BF16, isOutput=False)
    p_sel_he = nc.declare_dram_parameter("sel_he", [H, E], BF16, isOutput=False)
    p_fcwb = nc.declare_dram_parameter("fc_wb", [R + 1, NCLASS], F32, isOutput=False)
    p_out = nc.declare_dram_parameter("out", [BC, NCLASS], F32, isOutput=True)
    p_ur = nc.declare_dram_parameter("ur_dbg", [R, BC], F32, isOutput=True)

    with tile.TileContext(nc) as tc:
        with (
            tc.tile_pool(name="const", bufs=1) as cpool,
            tc.tile_pool(name="big", bufs=1) as bpool,
            tc.tile_pool(name="work", bufs=3) as wpool,
            tc.tile_pool(name="kwork", bufs=3) as kpool,
            tc.tile_pool(name="ps_att", bufs=3, space="PSUM") as pp_att,
            tc.tile_pool(name="ps_lg", bufs=2, space="PSUM") as pp_lg,
            tc.tile_pool(name="ps_tp", bufs=2, space="PSUM") as pp_tp,
            tc.tile_pool(name="ps_l2", bufs=1, space="PSUM") as pp_l2,
        ):
            # ---------- constants (loaded once, outside the repeat loop) ----------
            t_ident = cpool.tile([128, 128], F32)
            make_identity(nc, t_ident[:])
            t_user = cpool.tile([BC, 1], I32)
            nc.sync.dma_start(out=t_user[:], in_=p_user[:])
            t_nbg = cpool.tile([128, G], I32)
            nc.sync.dma_start(out=t_nbg[:], in_=p_nbg[:])
            t_seq = cpool.tile([128, G], I32)
            nc.sync.dma_start(out=t_seq[:], in_=p_seq[:])
            t_itg = cpool.tile([128, G], I32)
            nc.sync.dma_start(out=t_itg[:], in_=p_itg[:])
            t_negm = cpool.tile([1, J], BF16)
            nc.sync.dma_start(out=t_negm[:], in_=p_negm[:])
            t_wq = cpool.tile([E, E], F32)
            nc.sync.dma_start(out=t_wq[:], in_=p_wq[:])
            t_wk = cpool.tile([E, E], BF16)
            nc.sync.dma_start(out=t_wk[:], in_=p_wk[:])
            t_wv = cpool.tile([E, E], BF16)
            nc.sync.dma_start(out=t_wv[:], in_=p_wv[:])
            t_wo = cpool.tile([E, E], F32)
            nc.sync.dma_start(out=t_wo[:], in_=p_wo[:])
            t_bias = cpool.tile([E, 4], F32)
            nc.sync.dma_start(out=t_bias[:], in_=p_bias[:])
            t_sel_eh = cpool.tile([E, H], BF16)
            nc.sync.dma_start(out=t_sel_eh[:], in_=p_sel_eh[:])
            t_sel_he = cpool.tile([H, E], BF16)
            nc.sync.dma_start(out=t_sel_he[:], in_=p_sel_he[:])
            t_fcwb = cpool.tile([R + 1, NCLASS], F32)
            nc.sync.dma_start(out=t_fcwb[:], in_=p_fcwb[:])
            t_ones18 = cpool.tile([1, H], BF16)
            nc.vector.memset(t_ones18[:], 1.0)
            # per-pair padding mask (1.0 where neighbor > 0), [128, G]
            t_wcol = cpool.tile([128, G], F32)
            nc.vector.tensor_scalar(out=t_wcol[:], in0=t_nbg[:], scalar1=0,
                                    scalar2=None, op0=ALU.is_gt)

            def body():
                # user rows -> [BC, E] -> transpose -> uT [E, BC]
                t_u = wpool.tile([BC, E], F32, tag="gath_u")
                nc.gpsimd.indirect_dma_start(
                    out=t_u[:], out_offset=None, in_=p_uemb[:],
                    in_offset=bass.IndirectOffsetOnAxis(ap=t_user[:, :1], axis=0))
                ps_uT = pp_tp.tile([E, BC], F32, tag="tp")
                nc.tensor.transpose(out=ps_uT[:], in_=t_u[:], identity=t_ident[:BC, :BC])
                t_uT = bpool.tile([E, BC], F32)
                nc.scalar.copy(out=t_uT[:], in_=ps_uT[:])

                # qT = (wq.T @ uT + bq) * (1/sqrt(D))  -- scale folded in here
                ps_q = pp_tp.tile([E, BC], F32, tag="tp")
                nc.tensor.matmul(out=ps_q[:], lhsT=t_wq[:], rhs=t_uT[:])
                t_qT = bpool.tile([E, BC], F32)
                nc.scalar.activation(out=t_qT[:], in_=ps_q[:], func=AF.Identity,
                                     bias=t_bias[:, 0:1], scale=1.0 / np.sqrt(D))

                # neighbor rows, mega-gathered in 3 chunks
                t_nball = bpool.tile([128, J], F32)
                t_nbT = bpool.tile([E, J], BF16)
                t_kT = bpool.tile([E, J], BF16)
                t_vT = bpool.tile([E, J], F32)
                t_att = bpool.tile([H, J], BF16)
                t_oT = bpool.tile([E, J], F32)
                t_ratT = bpool.tile([R, J], BF16)
                # multiplier tile: left half ctxo*w (pair-major), right half item
                t_mul = bpool.tile([128, 2 * J], BF16)
                with tc.high_priority(offset=100000):
                    nc.gpsimd.indirect_dma_start(
                        out=t_mul[:, J:2 * J], out_offset=None, in_=p_item[:],
                        in_offset=bass.IndirectOffsetOnAxis(ap=t_itg[:, 0:G], axis=0))

                def emit_nb_chunk(c0, cg):
                    with tc.high_priority(offset=100000):
                        nc.gpsimd.indirect_dma_start(
                            out=t_nball[:, c0 * 128:(c0 + cg) * 128], out_offset=None,
                            in_=p_uemb[:],
                            in_offset=bass.IndirectOffsetOnAxis(ap=t_nbg[:, c0:c0 + cg],
                                                                axis=0))

                def emit_nb_group(g):
                    # neighbor rows for group g, transposed into nbT slice
                    with tc.high_priority(offset=50000):
                        ps_t = pp_tp.tile([128, 128], F32, tag="tp")
                        nc.tensor.transpose(out=ps_t[:],
                                            in_=t_nball[:, g * 128:(g + 1) * 128],
                                            identity=t_ident[:])
                        nc.scalar.copy(out=t_nbT[:, g * 128:(g + 1) * 128], in_=ps_t[:])

                def emit_attn_a(ci, c0, cn):
                    ctx_pri = tc.high_priority(offset=50000)
                    ctx_pri.__enter__()
                    sl = slice(c0, c0 + cn)
                    nb = cn // S
                    b0 = c0 // S
                    # k/v projections for this chunk
                    ps_k = pp_att.tile([E, 400], F32, tag="att")
                    nc.tensor.matmul(out=ps_k[:, :cn], lhsT=t_wk[:],
                                     rhs=t_nbT[:, sl])
                    nc.scalar.activation(out=t_kT[:, sl], in_=ps_k[:, :cn],
                                         func=AF.Identity, bias=t_bias[:, 1:2], scale=1.0)
                    ps_v = pp_att.tile([E, 400], F32, tag="att")
                    nc.tensor.matmul(out=ps_v[:, :cn], lhsT=t_wv[:],
                                     rhs=t_nbT[:, sl])
                    nc.scalar.activation(out=t_vT[:, sl], in_=ps_v[:, :cn],
                                         func=AF.Identity, bias=t_bias[:, 2:3], scale=1.0)
                    # prod_qk = kT * q_b (per-b Act pass; 1/sqrt(D) already in qT)
                    for bi in range(nb):
                        bsl = slice(c0 + bi * S, c0 + (bi + 1) * S)
                        nc.scalar.activation(out=t_kT[:, bsl], in_=t_kT[:, bsl],
                                             func=AF.Identity,
                                             scale=t_qT[:, b0 + bi:b0 + bi + 1])
                    # logits = per-head sums + negmask; attn = exp(logits)
                    # (masked entries underflow to exactly 0, matching the
                    # reference softmax); normalized by the row sum below.
                    ps_lg = pp_lg.tile([H, 400], F32, tag="lg")
                    nc.tensor.matmul(out=ps_lg[:, :cn], lhsT=t_sel_eh[:], rhs=t_kT[:, sl],
                                     start=True, stop=False)
                    nc.tensor.matmul(out=ps_lg[:, :cn], lhsT=t_ones18[:], rhs=t_negm[:, sl],
                                     start=False, stop=True)
                    t_ssc = wpool.tile([H, 8], F32, tag="ssc")
                    ss_tiles[ci] = t_ssc
                    for bi in range(cn // S):
                        nc.scalar.activation(out=t_att[:, c0 + bi * S:c0 + (bi + 1) * S],
                                             in_=ps_lg[:, bi * S:(bi + 1) * S], func=AF.Exp,
                                             accum_out=t_ssc[:, bi:bi + 1])
                    ctx_pri.__exit__(None, None, None)

                def emit_attn_b(ci, c0, cn):
                    ctx_pri = tc.high_priority(offset=50000)
                    ctx_pri.__enter__()
                    sl = slice(c0, c0 + cn)
                    nb = cn // S
                    t_ssc = ss_tiles.pop(ci)
                    t_rs = wpool.tile([H, 8], F32, tag="sm2")
                    nc.vector.reciprocal(out=t_rs[:, :nb], in_=t_ssc[:, :nb])
                    for bi in range(nb):
                        bsl = slice(c0 + bi * S, c0 + (bi + 1) * S)
                        nc.scalar.activation(out=t_att[:, bsl], in_=t_att[:, bsl],
                                             func=AF.Identity,
                                             scale=t_rs[:, bi:bi + 1])
                    # ctxT = attn_bcast * vT (in place over vT); ctxo = wo.T@ctx + bo
                    ps_ab = pp_att.tile([E, 400], F32, tag="att")
                    nc.tensor.matmul(out=ps_ab[:, :cn], lhsT=t_sel_he[:], rhs=t_att[:, sl])
                    nc.vector.tensor_tensor(out=t_vT[:, sl], in0=t_vT[:, sl],
                                            in1=ps_ab[:, :cn], op=ALU.mult)
                    ps_o = pp_att.tile([E, 400], F32, tag="att")
                    nc.tensor.matmul(out=ps_o[:, :cn], lhsT=t_wo[:], rhs=t_vT[:, sl])
                    nc.scalar.activation(out=t_oT[:, sl], in_=ps_o[:, :cn],
                                         func=AF.Identity, bias=t_bias[:, 3:4], scale=1.0)
                    ctx_pri.__exit__(None, None, None)

                k_tiles = {}
                ss_tiles = {}

                def emit_k_gather(u):
                    g0, ug = UNITS[u]
                    t_k = kpool.tile([128, ug * KROW], BF16, tag="k")
                    k_tiles[u] = t_k
                    with tc.high_priority(offset=100000):
                        nc.gpsimd.indirect_dma_start(
                            out=t_k[:], out_offset=None, in_=p_kcat[:],
                            in_offset=bass.IndirectOffsetOnAxis(ap=t_seq[:, g0:g0 + ug],
                                                                axis=0))

                def emit_mul_prep(g):
                    # ctxo (pair-major, scaled by the padding mask) -> mul left half
                    with tc.high_priority(offset=40000):
                        ps_tp = pp_tp.tile([128, 128], F32, tag="tp")
                        nc.tensor.transpose(out=ps_tp[:],
                                            in_=t_oT[:, g * 128:(g + 1) * 128],
                                            identity=t_ident[:])
                        nc.scalar.activation(out=t_mul[:, g * 128:(g + 1) * 128],
                                             in_=ps_tp[:], func=AF.Identity,
                                             scale=t_wcol[:, g:g + 1])

                def emit_unit_compute(u):
                    g0, ug = UNITS[u]
                    t_k = k_tiles.pop(u)
                    for gi in range(ug):
                        g = g0 + gi
                        mode = MODES[g]
                        off = gi * KROW
                        # product: K[t,m,r,e] *= mul[(t),m-permuted e] broadcast
                        # over r (mul halves: t=0 ctxo' at cols g*128.., t=1 item
                        # at J + g*128..)
                        k_in = _off(t_k[:], off,
                                    [[R * E, 2], [R * E // 2, 2], [E // 2, R],
                                     [1, E // 2]])
                        mul_in = _off(t_mul[:], g * 128,
                                      [[J, 2], [E // 2, 2], [0, R], [1, E // 2]])
                        nc.vector.tensor_tensor(out=k_in, in0=k_in, in1=mul_in,
                                                op=ALU.mult)
                        # reduce over e via halving folds; layout per group chunk
                        # is (t, m, r, e_low): fold1 halves m, later folds e_low
                        km = lambda h: _off(t_k[:], off,
                                            [[R * E, 2], [E // 2, R], [1, h]])
                        kmh = lambda h: _off(t_k[:], off + h,
                                             [[R * E, 2], [E // 2, R], [1, h]])
                        # fold1: m=0 half += m=1 half (contiguous 2048-elem chunks)
                        f1_out = _off(t_k[:], off, [[R * E, 2], [1, R * E // 2]])
                        f1_in = _off(t_k[:], off + R * E // 2,
                                     [[R * E, 2], [1, R * E // 2]])
                        if mode in ('b', 'c'):
                            nc.gpsimd.dma_start(out=f1_out, in_=f1_in,
                                                accum_op=ALU.add)
                        else:
                            nc.vector.tensor_tensor(out=f1_out, in0=f1_out,
                                                    in1=f1_in, op=ALU.add)
                        eng = nc.gpsimd if mode in ('c', 'e') else nc.vector
                        w = E // 2
                        while w > 1:
                            h = w // 2
                            eng.tensor_tensor(out=km(h), in0=km(h), in1=kmh(h),
                                              op=ALU.add)
                            w = h
                    # rating = nprof' * iprof (w already folded into ctxo side)
                    t_rat = wpool.tile([128, ug * R], F32, tag="rat")
                    rat_n = _ap(t_k[:], [[KROW, ug], [E // 2, R]])
                    rat_i = _off(t_k[:], R * E, [[KROW, ug], [E // 2, R]])
                    nc.vector.tensor_tensor(out=t_rat[:], in0=rat_n, in1=rat_i,
                                            op=ALU.mult)
                    # transpose [128, ug*R] -> [ug*R, 128] -> ratT rows
                    ps_rt = pp_tp.tile([2 * R, 128], F32, tag="tp")
                    nc.tensor.transpose(out=ps_rt[:ug * R, :], in_=t_rat[:],
                                        identity=t_ident[:])
                    for gi in range(ug):
                        g = g0 + gi
                        nc.scalar.copy(out=t_ratT[:, g * 128:(g + 1) * 128],
                                       in_=ps_rt[gi * R:(gi + 1) * R, :])

                # software-pipelined emission: neighbor transposes -> attention
                # stage A -> K-unit computes -> attention stage B -> mul preps
                nbc = 0
                g_nb = 0
                g_kg = 0
                g_prep = 0
                u_comp = 0
                for ci, (c0, cn) in enumerate(CH400):
                    hi = c0 + cn
                    while nbc < len(NB_CHUNKS) and NB_CHUNKS[nbc][0] * 128 < hi:
                        emit_nb_chunk(*NB_CHUNKS[nbc])
                        nbc += 1
                    while g_nb * 128 < hi:
                        emit_nb_group(g_nb)
                        g_nb += 1
                    while g_kg < len(UNITS) and UNITS[g_kg][0] * 128 <= hi + KPREFETCH * 256:
                        emit_k_gather(g_kg)
                        g_kg += 1
                    emit_attn_a(ci, c0, cn)
                    while u_comp < len(UNITS):
                        g0, ug = UNITS[u_comp]
                        if (g0 + ug) > g_prep - 1:
                            break
                        emit_unit_compute(u_comp)
                        u_comp += 1
                    emit_attn_b(ci, c0, cn)
                    while (g_prep + 1) * 128 <= hi:
                        emit_mul_prep(g_prep)
                        g_prep += 1
                while g_kg < len(UNITS):
                    emit_k_gather(g_kg)
                    g_kg += 1
                while g_prep < G:
                    emit_mul_prep(g_prep)
                    g_prep += 1
                while u_comp < len(UNITS):
                    emit_unit_compute(u_comp)
                    u_comp += 1

                # user rating vector: max over s
                t_urp = wpool.tile([R + 1, BC], F32, tag="urp")
                nc.vector.tensor_reduce(out=t_urp[:R, :],
                                        in_=t_ratT[:].rearrange("r (b s) -> r b s", s=S),
                                        axis=mybir.AxisListType.X, op=ALU.max)
                nc.vector.memset(t_urp[R:R + 1, :], 1.0)
                nc.sync.dma_start(out=p_ur[:], in_=t_urp[:R, :])

                # classifier + softmax
                ps_l2 = pp_l2.tile([BC, NCLASS], F32, tag="l2")
                nc.tensor.matmul(out=ps_l2[:], lhsT=t_urp[:], rhs=t_fcwb[:])
                t_nm2 = wpool.tile([BC, 1], F32, tag="fin")
                nc.vector.tensor_reduce(out=t_nm2[:], in_=ps_l2[:],
                                        axis=mybir.AxisListType.X,
                                        op=ALU.max, negate=True)
                t_e2 = wpool.tile([BC, NCLASS], F32, tag="fin2")
                t_s2 = wpool.tile([BC, 1], F32, tag="fin3")
                nc.scalar.activation(out=t_e2[:], in_=ps_l2[:], func=AF.Exp,
                                     bias=t_nm2[:, :1], scale=1.0,
                                     accum_out=t_s2[:, :1])
                t_r2 = wpool.tile([BC, 1], F32, tag="fin4")
                nc.vector.reciprocal(out=t_r2[:], in_=t_s2[:])
                t_o = wpool.tile([BC, NCLASS], F32, tag="fin5")
                nc.vector.tensor_scalar(out=t_o[:], in0=t_e2[:], scalar1=t_r2[:, :1],
                                        scalar2=None, op0=ALU.mult)
                nc.sync.dma_start(out=p_out[:], in_=t_o[:])

            if repeat == 1:
                body()
            else:
                with tc.For_i(0, repeat, 1):
                    body()

    nc.finalize()
    return nc


def prep_in_maps(inputs):
    user = np.asarray(inputs["user"]).astype(np.int32).reshape(B)
    item = np.asarray(inputs["item"]).astype(np.int32).reshape(B)
    neighbor = np.asarray(inputs["neighbor"]).astype(np.int32).reshape(B, S)
    seq = np.asarray(inputs["seq"]).astype(np.int32).reshape(B, S)

    f32 = lambda x: np.ascontiguousarray(np.asarray(x, dtype=np.float32))
    bf16 = lambda x: np.ascontiguousarray(
        np.asarray(x, dtype=np.float32).astype(ml_dtypes.bfloat16))

    uemb = f32(inputs["user_emb_table"])
    item_bf = bf16(inputs["item_emb_table"])
    # both LCU tables in one row: (t, e_msb, r, e_low) so the first fold's
    # halves are contiguous 2048-elem chunks per table
    perm = lambda t: t.reshape(ALLSEQ, R, 2, E // 2).transpose(0, 2, 1, 3).reshape(
        ALLSEQ, R * E)
    lcu_cat = np.ascontiguousarray(np.concatenate(
        [perm(bf16(inputs["item_user_LCU"])),
         perm(bf16(inputs["user_item_LCU"]))], axis=1))
    biases = np.ascontiguousarray(np.stack(
        [f32(inputs["bq"]), f32(inputs["bk"]),
         f32(inputs["bv"]), f32(inputs["bo"])], axis=1))
    sel_eh = np.zeros((E, H), np.float32)
    sel_eh[np.arange(E), np.arange(E) // D] = 1.0
    fc_wb = np.ascontiguousarray(np.concatenate(
        [f32(inputs["fc_w"]), f32(inputs["fc_b"]).reshape(1, NCLASS)], axis=0))

    shared = {
        "uemb": uemb, "item_bf": item_bf, "lcu_cat": lcu_cat,
        "wq": f32(inputs["wq"]), "wk": bf16(inputs["wk"]),
        "wv": bf16(inputs["wv"]), "wo": f32(inputs["wo"]),
        "biases": biases, "sel_eh": np.ascontiguousarray(sel_eh.astype(ml_dtypes.bfloat16)),
        "sel_he": np.ascontiguousarray(sel_eh.T.astype(ml_dtypes.bfloat16)), "fc_wb": fc_wb,
    }
    in_maps = []
    for c in range(NCORES):
        bsl = slice(c * BC, (c + 1) * BC)
        nb = neighbor[bsl].reshape(J)
        sq = seq[bsl].reshape(J)
        itx = np.repeat(item[bsl], S)
        col = lambda x: np.ascontiguousarray(x.reshape(G, 128).T.astype(np.int32))
        in_maps.append({
            **shared,
            "user_i": np.ascontiguousarray(user[bsl].reshape(BC, 1)),
            "nbg_i": col(nb), "seq_i": col(sq), "itg_i": col(itx),
            "negmask": np.ascontiguousarray(
                (-1e9 * (nb <= 0)).astype(ml_dtypes.bfloat16).reshape(1, J)),
        })
    return in_maps


_NC_CACHE = {}


def kernel(**inputs):
    if "nc" not in _NC_CACHE:
        _NC_CACHE["nc"] = build_nc(repeat=1)
    nc = _NC_CACHE["nc"]
    in_maps = prep_in_maps(inputs)
    res = run_bass_kernel_spmd(nc, in_maps, core_ids=list(range(NCORES)))
    return np.concatenate([res.results[c]["out"] for c in range(NCORES)], axis=0)


# revision 18
# speedup vs baseline: 1.1250x; 1.0168x over previous
"""ARERec forward kernel for 8 TRN2 NeuronCores.

Data-parallel over batch: each core processes B/8 = 64 batch rows end-to-end
(embedding gathers, single-query multi-head attention, LCU region profiles,
rating classifier); tables and weights are replicated. The final [512, 5]
softmax output is concatenated on the host from the 8 per-core [64, 5] shards.

Engine balance (per core, cost-model ns): the serial DMA bus moves ~52MB of
LCU rows (~150us floor); DVE holds the region-profile products; the fold
chains that reduce each product over e are split between DVE, DMA-accumulate
(CCE add) and Pool (gpsimd) so no single engine exceeds the DMA floor by
much. Both LCU tables are host-merged into one [ALLSEQ, 2*R*E] row so each
128-pair group needs one gather; item/neighbor rows use one mega-gather each.

Self-contained: shapes/sharding are hardcoded from the problem spec.
"""
import numpy as np
import ml_dtypes

import concourse.bacc as bacc
import concourse.bass as bass
import concourse.mybir as mybir
import concourse.tile as tile
from concourse.masks import make_identity
from concourse.bass_utils import run_bass_kernel_spmd

NCORES = 8
B, S, E, H, R = 512, 50, 128, 8, 32
D = E // H
USERS, ITEMS, ALLSEQ, NCLASS = 50000, 20000, 20000, 5
BC = B // NCORES            # 64 batch rows per core
J = BC * S                  # 3200 (b, s) pairs per core
G = J // 128                # 25 gather groups of 128 pairs
KROW = 2 * R * E            # 8192 combined-table row (kiu | kui)
# b-aligned chunks (multiples of S); two small priming chunks let the
# attention->profile software pipeline fill faster
CH400 = [(0, 200), (200, 200)] + [(c * 400, 400) for c in range(1, J // 400)]
# K-gather units: one group per indirect gather (finer DMA granularity keeps
# the serial DMA bus fed and more groups in flight)
UNITS = [(g, 1) for g in range(G)]
# reduce mode per group: 'a' all-DVE chain, 'b' fold1-DMA + DVE rest,
# 'c' fold1-DMA + Pool rest, 'e' fold1-DVE + Pool rest
MODES = ['e', 'b', 'e', 'a', 'e', 'b', 'e', 'e', 'b', 'a', 'e', 'b', 'e',
         'e', 'b', 'a', 'e', 'b', 'e', 'e', 'b', 'a', 'e', 'c', 'c']
NB_CHUNKS = [(0, 9), (9, 8), (17, 8)]   # neighbor mega-gather splits
KPREFETCH = 6               # groups of K gather issued ahead of compute

F32 = mybir.dt.float32
BF16 = mybir.dt.bfloat16
I32 = mybir.dt.int32
AF = mybir.ActivationFunctionType
ALU = mybir.AluOpType


def _ap(ap, dims):
    """Rebuild an AP with explicit [step, count] free dims (partition dim kept)."""
    return bass.AP(tensor=ap.tensor, offset=ap.offset, ap=[ap.ap[0]] + dims)


def _off(ap, extra_offset, dims):
    return bass.AP(tensor=ap.tensor, offset=ap.offset + extra_offset,
                   ap=[ap.ap[0]] + dims)


def build_nc(repeat=1):
    nc = bacc.Bacc(None, target_bir_lowering=False)

    p_user = nc.declare_dram_parameter("user_i", [BC, 1], I32, isOutput=False)
    p_nbg = nc.declare_dram_parameter("nbg_i", [128, G], I32, isOutput=False)
    p_seq = nc.declare_dram_parameter("seq_i", [128, G], I32, isOutput=False)
    p_itg = nc.declare_dram_parameter("itg_i", [128, G], I32, isOutput=False)
    p_negm = nc.declare_dram_parameter("negmask", [1, J], BF16, isOutput=False)
    p_uemb = nc.declare_dram_parameter("uemb", [USERS, E], F32, isOutput=False)
    p_item = nc.declare_dram_parameter("item_bf", [ITEMS, E], BF16, isOutput=False)
    p_kcat = nc.declare_dram_parameter("lcu_cat", [ALLSEQ, KROW], BF16, isOutput=False)
    p_wq = nc.declare_dram_parameter("wq", [E, E], F32, isOutput=False)
    p_wk = nc.declare_dram_parameter("wk", [E, E], BF16, isOutput=False)
    p_wv = nc.declare_dram_parameter("wv", [E, E], BF16, isOutput=False)
    p_wo = nc.declare_dram_parameter("wo", [E, E], F32, isOutput=False)
    p_bias = nc.declare_dram_parameter("biases", [E, 4], F32, isOutput=False)
    p_sel_eh = nc.declare_dram_parameter("sel_eh", [E, H], BF16, isOutput=False)
    p_sel_he = nc.declare_dram_parameter("sel_he", [H, E], BF16, isOutput=False)
    p_fcwb = nc.declare_dram_parameter("fc_wb", [R + 1, NCLASS], F32, isOutput=False)
    p_out = nc.declare_dram_parameter("out", [BC, NCLASS], F32, isOutput=True)
    p_ur = nc.declare_dram_parameter("ur_dbg", [R, BC], F32, isOutput=True)

    with tile.TileContext(nc) as tc:
        with (
            tc.tile_pool(name="const", bufs=1) as cpool,
            tc.tile_pool(name="big", bufs=1) as bpool,
            tc.tile_pool(name="work", bufs=3) as wpool,
            tc.tile_pool(name="kwork", bufs=6) as kpool,
            tc.tile_pool(name="ps_att", bufs=3, space="PSUM") as pp_att,
            tc.tile_pool(name="ps_lg", bufs=2, space="PSUM") as pp_lg,
            tc.tile_pool(name="ps_tp", bufs=2, space="PSUM") as pp_tp,
            tc.tile_pool(name="ps_l2", bufs=1, space="PSUM") as pp_l2,
        ):
            # ---------- constants (loaded once, outside the repeat loop) ----------
            t_ident = cpool.tile([128, 128], F32)
            make_identity(nc, t_ident[:])
            t_user = cpool.tile([BC, 1], I32)
            nc.sync.dma_start(out=t_user[:], in_=p_user[:])
            t_nbg = cpool.tile([128, G], I32)
            nc.sync.dma_start(out=t_nbg[:], in_=p_nbg[:])
            t_seq = cpool.tile([128, G], I32)
            nc.sync.dma_start(out=t_seq[:], in_=p_seq[:])
            t_itg = cpool.tile([128, G], I32)
            nc.sync.dma_start(out=t_itg[:], in_=p_itg[:])
            t_negm = cpool.tile([1, J], BF16)
            nc.sync.dma_start(out=t_negm[:], in_=p_negm[:])
            t_wq = cpool.tile([E, E], F32)
            nc.sync.dma_start(out=t_wq[:], in_=p_wq[:])
            t_wk = cpool.tile([E, E], BF16)
            nc.sync.dma_start(out=t_wk[:], in_=p_wk[:])
            t_wv = cpool.tile([E, E], BF16)
            nc.sync.dma_start(out=t_wv[:], in_=p_wv[:])
            t_wo = cpool.tile([E, E], F32)
            nc.sync.dma_start(out=t_wo[:], in_=p_wo[:])
            t_bias = cpool.tile([E, 4], F32)
            nc.sync.dma_start(out=t_bias[:], in_=p_bias[:])
            t_sel_eh = cpool.tile([E, H], BF16)
            nc.sync.dma_start(out=t_sel_eh[:], in_=p_sel_eh[:])
            t_sel_he = cpool.tile([H, E], BF16)
            nc.sync.dma_start(out=t_sel_he[:], in_=p_sel_he[:])
            t_fcwb = cpool.tile([R + 1, NCLASS], F32)
            nc.sync.dma_start(out=t_fcwb[:], in_=p_fcwb[:])
            t_ones18 = cpool.tile([1, H], BF16)
            nc.vector.memset(t_ones18[:], 1.0)
            # per-pair padding mask (1.0 where neighbor > 0), [128, G]
            t_wcol = cpool.tile([128, G], F32)
            nc.vector.tensor_scalar(out=t_wcol[:], in0=t_nbg[:], scalar1=0,
                                    scalar2=None, op0=ALU.is_gt)

            def body():
                # user rows -> [BC, E] -> transpose -> uT [E, BC]
                t_u = wpool.tile([BC, E], F32, tag="gath_u")
                nc.gpsimd.indirect_dma_start(
                    out=t_u[:], out_offset=None, in_=p_uemb[:],
                    in_offset=bass.IndirectOffsetOnAxis(ap=t_user[:, :1], axis=0))
                ps_uT = pp_tp.tile([E, BC], F32, tag="tp")
                nc.tensor.transpose(out=ps_uT[:], in_=t_u[:], identity=t_ident[:BC, :BC])
                t_uT = bpool.tile([E, BC], F32)
                nc.scalar.copy(out=t_uT[:], in_=ps_uT[:])

                # qT = (wq.T @ uT + bq) * (1/sqrt(D))  -- scale folded in here
                ps_q = pp_tp.tile([E, BC], F32, tag="tp")
                nc.tensor.matmul(out=ps_q[:], lhsT=t_wq[:], rhs=t_uT[:])
                t_qT = bpool.tile([E, BC], F32)
                nc.scalar.activation(out=t_qT[:], in_=ps_q[:], func=AF.Identity,
                                     bias=t_bias[:, 0:1], scale=1.0 / np.sqrt(D))

                # neighbor rows, mega-gathered in 3 chunks
                t_nball = bpool.tile([128, J], F32)
                t_nbT = bpool.tile([E, J], BF16)
                t_kT = bpool.tile([E, J], BF16)
                t_vT = bpool.tile([E, J], F32)
                t_att = bpool.tile([H, J], BF16)
                t_oT = bpool.tile([E, J], F32)
                t_ratT = bpool.tile([R, J], BF16)
                # multiplier tile: left half ctxo*w (pair-major), right half item
                t_mul = bpool.tile([128, 2 * J], BF16)
                with tc.high_priority(offset=100000):
                    nc.gpsimd.indirect_dma_start(
                        out=t_mul[:, J:2 * J], out_offset=None, in_=p_item[:],
                        in_offset=bass.IndirectOffsetOnAxis(ap=t_itg[:, 0:G], axis=0))

                def emit_nb_chunk(c0, cg):
                    with tc.high_priority(offset=100000):
                        nc.gpsimd.indirect_dma_start(
                            out=t_nball[:, c0 * 128:(c0 + cg) * 128], out_offset=None,
                            in_=p_uemb[:],
                            in_offset=bass.IndirectOffsetOnAxis(ap=t_nbg[:, c0:c0 + cg],
                                                                axis=0))

                def emit_nb_group(g):
                    # neighbor rows for group g, transposed into nbT slice
                    with tc.high_priority(offset=50000):
                        ps_t = pp_tp.tile([128, 128], F32, tag="tp")
                        nc.tensor.transpose(out=ps_t[:],
                                            in_=t_nball[:, g * 128:(g + 1) * 128],
                                            identity=t_ident[:])
                        nc.scalar.copy(out=t_nbT[:, g * 128:(g + 1) * 128], in_=ps_t[:])

                def emit_attn_a(ci, c0, cn):
                    ctx_pri = tc.high_priority(offset=50000)
                    ctx_pri.__enter__()
                    sl = slice(c0, c0 + cn)
                    nb = cn // S
                    b0 = c0 // S
                    # k/v projections for this chunk
                    ps_k = pp_att.tile([E, 400], F32, tag="att")
                    nc.tensor.matmul(out=ps_k[:, :cn], lhsT=t_wk[:],
                                     rhs=t_nbT[:, sl])
                    nc.scalar.activation(out=t_kT[:, sl], in_=ps_k[:, :cn],
                                         func=AF.Identity, bias=t_bias[:, 1:2], scale=1.0)
                    ps_v = pp_att.tile([E, 400], F32, tag="att")
                    nc.tensor.matmul(out=ps_v[:, :cn], lhsT=t_wv[:],
                                     rhs=t_nbT[:, sl])
                    nc.scalar.activation(out=t_vT[:, sl], in_=ps_v[:, :cn],
                                         func=AF.Identity, bias=t_bias[:, 2:3], scale=1.0)
                    # prod_qk = kT * q_b (per-b Act pass; 1/sqrt(D) already in qT)
                    for bi in range(nb):
                        bsl = slice(c0 + bi * S, c0 + (bi + 1) * S)
                        nc.scalar.activation(out=t_kT[:, bsl], in_=t_kT[:, bsl],
                                             func=AF.Identity,
                                             scale=t_qT[:, b0 + bi:b0 + bi + 1])
                    # logits = per-head sums + negmask; attn = exp(logits)
                    # (masked entries underflow to exactly 0, matching the
                    # reference softmax); normalized by the row sum below.
                    ps_lg = pp_lg.tile([H, 400], F32, tag="lg")
                    nc.tensor.matmul(out=ps_lg[:, :cn], lhsT=t_sel_eh[:], rhs=t_kT[:, sl],
                                     start=True, stop=False)
                    nc.tensor.matmul(out=ps_lg[:, :cn], lhsT=t_ones18[:], rhs=t_negm[:, sl],
                                     start=False, stop=True)
                    t_ssc = wpool.tile([H, 8], F32, tag="ssc")
                    ss_tiles[ci] = t_ssc
                    for bi in range(cn // S):
                        nc.scalar.activation(out=t_att[:, c0 + bi * S:c0 + (bi + 1) * S],
                                             in_=ps_lg[:, bi * S:(bi + 1) * S], func=AF.Exp,
                                             accum_out=t_ssc[:, bi:bi + 1])
                    ctx_pri.__exit__(None, None, None)

                def emit_attn_b(ci, c0, cn):
                    ctx_pri = tc.high_priority(offset=50000)
                    ctx_pri.__enter__()
                    sl = slice(c0, c0 + cn)
                    nb = cn // S
                    t_ssc = ss_tiles.pop(ci)
                    t_rs = wpool.tile([H, 8], F32, tag="sm2")
                    nc.vector.reciprocal(out=t_rs[:, :nb], in_=t_ssc[:, :nb])
                    for bi in range(nb):
                        bsl = slice(c0 + bi * S, c0 + (bi + 1) * S)
                        nc.scalar.activation(out=t_att[:, bsl], in_=t_att[:, bsl],
                                             func=AF.Identity,
                                             scale=t_rs[:, bi:bi + 1])
                    # ctxT = attn_bcast * vT (in place over vT); ctxo = wo.T@ctx + bo
                    ps_ab = pp_att.tile([E, 400], F32, tag="att")
                    nc.tensor.matmul(out=ps_ab[:, :cn], lhsT=t_sel_he[:], rhs=t_att[:, sl])
                    nc.vector.tensor_tensor(out=t_vT[:, sl], in0=t_vT[:, sl],
                                            in1=ps_ab[:, :cn], op=ALU.mult)
                    ps_o = pp_att.tile([E, 400], F32, tag="att")
                    nc.tensor.matmul(out=ps_o[:, :cn], lhsT=t_wo[:], rhs=t_vT[:, sl])
                    nc.scalar.activation(out=t_oT[:, sl], in_=ps_o[:, :cn],
                                         func=AF.Identity, bias=t_bias[:, 3:4], scale=1.0)
                    ctx_pri.__exit__(None, None, None)

                k_tiles = {}
                ss_tiles = {}

                def emit_k_gather(u):
                    g0, ug = UNITS[u]
                    t_k = kpool.tile([128, ug * KROW], BF16, tag="k")
                    k_tiles[u] = t_k
                    with tc.high_priority(offset=100000):
                        nc.gpsimd.indirect_dma_start(
                            out=t_k[:], out_offset=None, in_=p_kcat[:],
                            in_offset=bass.IndirectOffsetOnAxis(ap=t_seq[:, g0:g0 + ug],
                                                                axis=0))

                def emit_mul_prep(g):
                    # ctxo (pair-major, scaled by the padding mask) -> mul left half
                    with tc.high_priority(offset=40000):
                        ps_tp = pp_tp.tile([128, 128], F32, tag="tp")
                        nc.tensor.transpose(out=ps_tp[:],
                                            in_=t_oT[:, g * 128:(g + 1) * 128],
                                            identity=t_ident[:])
                        nc.scalar.activation(out=t_mul[:, g * 128:(g + 1) * 128],
                                             in_=ps_tp[:], func=AF.Identity,
                                             scale=t_wcol[:, g:g + 1])

                def emit_unit_compute(u):
                    g0, ug = UNITS[u]
                    t_k = k_tiles.pop(u)
                    for gi in range(ug):
                        g = g0 + gi
                        mode = MODES[g]
                        off = gi * KROW
                        # product: K[t,m,r,e] *= mul[(t),m-permuted e] broadcast
                        # over r (mul halves: t=0 ctxo' at cols g*128.., t=1 item
                        # at J + g*128..)
                        k_in = _off(t_k[:], off,
                                    [[R * E, 2], [R * E // 2, 2], [E // 2, R],
                                     [1, E // 2]])
                        mul_in = _off(t_mul[:], g * 128,
                                      [[J, 2], [E // 2, 2], [0, R], [1, E // 2]])
                        nc.vector.tensor_tensor(out=k_in, in0=k_in, in1=mul_in,
                                                op=ALU.mult)
                        # reduce over e via halving folds; layout per group chunk
                        # is (t, m, r, e_low): fold1 halves m, later folds e_low
                        km = lambda h: _off(t_k[:], off,
                                            [[R * E, 2], [E // 2, R], [1, h]])
                        kmh = lambda h: _off(t_k[:], off + h,
                                             [[R * E, 2], [E // 2, R], [1, h]])
                        # fold1: m=0 half += m=1 half (contiguous 2048-elem chunks)
                        f1_out = _off(t_k[:], off, [[R * E, 2], [1, R * E // 2]])
                        f1_in = _off(t_k[:], off + R * E // 2,
                                     [[R * E, 2], [1, R * E // 2]])
                        if mode in ('b', 'c'):
                            nc.gpsimd.dma_start(out=f1_out, in_=f1_in,
                                                accum_op=ALU.add)
                        else:
                            nc.vector.tensor_tensor(out=f1_out, in0=f1_out,
                                                    in1=f1_in, op=ALU.add)
                        eng = nc.gpsimd if mode in ('c', 'e') else nc.vector
                        w = E // 2
                        while w > 1:
                            h = w // 2
                            eng.tensor_tensor(out=km(h), in0=km(h), in1=kmh(h),
                                              op=ALU.add)
                            w = h
                    # rating = nprof' * iprof (w already folded into ctxo side)
                    t_rat = wpool.tile([128, ug * R], F32, tag="rat")
                    rat_n = _ap(t_k[:], [[KROW, ug], [E // 2, R]])
                    rat_i = _off(t_k[:], R * E, [[KROW, ug], [E // 2, R]])
                    nc.vector.tensor_tensor(out=t_rat[:], in0=rat_n, in1=rat_i,
                                            op=ALU.mult)
                    # transpose [128, ug*R] -> [ug*R, 128] -> ratT rows
                    ps_rt = pp_tp.tile([2 * R, 128], F32, tag="tp")
                    nc.tensor.transpose(out=ps_rt[:ug * R, :], in_=t_rat[:],
                                        identity=t_ident[:])
                    for gi in range(ug):
                        g = g0 + gi
                        nc.scalar.copy(out=t_ratT[:, g * 128:(g + 1) * 128],
                                       in_=ps_rt[gi * R:(gi + 1) * R, :])

                # software-pipelined emission: neighbor transposes -> attention
                # stage A -> K-unit computes -> attention stage B -> mul preps
                nbc = 0
                g_nb = 0
                g_kg = 0
                g_prep = 0
                u_comp = 0
                for ci, (c0, cn) in enumerate(CH400):
                    hi = c0 + cn
                    while nbc < len(NB_CHUNKS) and NB_CHUNKS[nbc][0] * 128 < hi:
                        emit_nb_chunk(*NB_CHUNKS[nbc])
                        nbc += 1
                    while g_nb * 128 < hi:
                        emit_nb_group(g_nb)
                        g_nb += 1
                    while g_kg < len(UNITS) and UNITS[g_kg][0] * 128 <= hi + KPREFETCH * 128:
                        emit_k_gather(g_kg)
                        g_kg += 1
                    emit_attn_a(ci, c0, cn)
                    while u_comp < len(UNITS):
                        g0, ug = UNITS[u_comp]
                        if (g0 + ug) > g_prep - 1:
                            break
                        emit_unit_compute(u_comp)
                        u_comp += 1
                    emit_attn_b(ci, c0, cn)
                    while (g_prep + 1) * 128 <= hi:
                        emit_mul_prep(g_prep)
                        g_prep += 1
                while g_kg < len(UNITS):
                    emit_k_gather(g_kg)
                    g_kg += 1
                while g_prep < G:
                    emit_mul_prep(g_prep)
                    g_prep += 1
                while u_comp < len(UNITS):
                    emit_unit_compute(u_comp)
                    u_comp += 1

                # user rating vector: max over s
                t_urp = wpool.tile([R + 1, BC], F32, tag="urp")
                nc.vector.tensor_reduce(out=t_urp[:R, :],
                                        in_=t_ratT[:].rearrange("r (b s) -> r b s", s=S),
                                        axis=mybir.AxisListType.X, op=ALU.max)
                nc.vector.memset(t_urp[R:R + 1, :], 1.0)
                nc.sync.dma_start(out=p_ur[:], in_=t_urp[:R, :])

                # classifier + softmax
                ps_l2 = pp_l2.tile([BC, NCLASS], F32, tag="l2")
                nc.tensor.matmul(out=ps_l2[:], lhsT=t_urp[:], rhs=t_fcwb[:])
                t_nm2 = wpool.tile([BC, 1], F32, tag="fin")
                nc.vector.tensor_reduce(out=t_nm2[:], in_=ps_l2[:],
                                        axis=mybir.AxisListType.X,
                                        op=ALU.max, negate=True)
                t_e2 = wpool.tile([BC, NCLASS], F32, tag="fin2")
                t_s2 = wpool.tile([BC, 1], F32, tag="fin3")
                nc.scalar.activation(out=t_e2[:], in_=ps_l2[:], func=AF.Exp,
                                     bias=t_nm2[:, :1], scale=1.0,
                                     accum_out=t_s2[:, :1])
                t_r2 = wpool.tile([BC, 1], F32, tag="fin4")
                nc.vector.reciprocal(out=t_r2[:], in_=t_s2[:])
                t_o = wpool.tile([BC, NCLASS], F32, tag="fin5")
                nc.vector.tensor_scalar(out=t_o[:], in0=t_e2[:], scalar1=t_r2[:, :1],
                                        scalar2=None, op0=ALU.mult)
                nc.sync.dma_start(out=p_out[:], in_=t_o[:])

            if repeat == 1:
                body()
            else:
                with tc.For_i(0, repeat, 1):
                    body()

    nc.finalize()
    return nc


def prep_in_maps(inputs):
    user = np.asarray(inputs["user"]).astype(np.int32).reshape(B)
    item = np.asarray(inputs["item"]).astype(np.int32).reshape(B)
    neighbor = np.asarray(inputs["neighbor"]).astype(np.int32).reshape(B, S)
    seq = np.asarray(inputs["seq"]).astype(np.int32).reshape(B, S)

    f32 = lambda x: np.ascontiguousarray(np.asarray(x, dtype=np.float32))
    bf16 = lambda x: np.ascontiguousarray(
        np.asarray(x, dtype=np.float32).astype(ml_dtypes.bfloat16))

    uemb = f32(inputs["user_emb_table"])
    item_bf = bf16(inputs["item_emb_table"])
    # both LCU tables in one row: (t, e_msb, r, e_low) so the first fold's
    # halves are contiguous 2048-elem chunks per table
    perm = lambda t: t.reshape(ALLSEQ, R, 2, E // 2).transpose(0, 2, 1, 3).reshape(
        ALLSEQ, R * E)
    lcu_cat = np.ascontiguousarray(np.concatenate(
        [perm(bf16(inputs["item_user_LCU"])),
         perm(bf16(inputs["user_item_LCU"]))], axis=1))
    biases = np.ascontiguousarray(np.stack(
        [f32(inputs["bq"]), f32(inputs["bk"]),
         f32(inputs["bv"]), f32(inputs["bo"])], axis=1))
    sel_eh = np.zeros((E, H), np.float32)
    sel_eh[np.arange(E), np.arange(E) // D] = 1.0
    fc_wb = np.ascontiguousarray(np.concatenate(
        [f32(inputs["fc_w"]), f32(inputs["fc_b"]).reshape(1, NCLASS)], axis=0))

    shared = {
        "uemb": uemb, "item_bf": item_bf, "lcu_cat": lcu_cat,
        "wq": f32(inputs["wq"]), "wk": bf16(inputs["wk"]),
        "wv": bf16(inputs["wv"]), "wo": f32(inputs["wo"]),
        "biases": biases, "sel_eh": np.ascontiguousarray(sel_eh.astype(ml_dtypes.bfloat16)),
        "sel_he": np.ascontiguousarray(sel_eh.T.astype(ml_dtypes.bfloat16)), "fc_wb": fc_wb,
    }
    in_maps = []
    for c in range(NCORES):
        bsl = slice(c * BC, (c + 1) * BC)
        nb = neighbor[bsl].reshape(J)
        sq = seq[bsl].reshape(J)
        itx = np.repeat(item[bsl], S)
        col = lambda x: np.ascontiguousarray(x.reshape(G, 128).T.astype(np.int32))
        in_maps.append({
            **shared,
            "user_i": np.ascontiguousarray(user[bsl].reshape(BC, 1)),
            "nbg_i": col(nb), "seq_i": col(sq), "itg_i": col(itx),
            "negmask": np.ascontiguousarray(
                (-1e9 * (nb <= 0)).astype(ml_dtypes.bfloat16).reshape(1, J)),
        })
    return in_maps


_NC_CACHE = {}


def kernel(**inputs):
    if "nc" not in _NC_CACHE:
        _NC_CACHE["nc"] = build_nc(repeat=1)
    nc = _NC_CACHE["nc"]
    in_maps = prep_in_maps(inputs)
    res = run_bass_kernel_spmd(nc, in_maps, core_ids=list(range(NCORES)))
    return np.concatenate([res.results[c]["out"] for c in range(NCORES)], axis=0)


# revision 21
# speedup vs baseline: 1.1454x; 1.0181x over previous
"""ARERec forward kernel for 8 TRN2 NeuronCores.

Data-parallel over batch: each core processes B/8 = 64 batch rows end-to-end
(embedding gathers, single-query multi-head attention, LCU region profiles,
rating classifier); tables and weights are replicated. The final [512, 5]
softmax output is concatenated on the host from the 8 per-core [64, 5] shards.

Engine balance (per core, cost-model ns): the serial DMA bus moves ~52MB of
LCU rows (~150us floor); DVE holds the region-profile products; the fold
chains that reduce each product over e are split between DVE, DMA-accumulate
(CCE add) and Pool (gpsimd) so no single engine exceeds the DMA floor by
much. Both LCU tables are host-merged into one [ALLSEQ, 2*R*E] row so each
128-pair group needs one gather; item/neighbor rows use one mega-gather each.

Self-contained: shapes/sharding are hardcoded from the problem spec.
"""
import numpy as np
import ml_dtypes

import concourse.bacc as bacc
import concourse.bass as bass
import concourse.mybir as mybir
import concourse.tile as tile
from concourse.masks import make_identity
from concourse.bass_utils import run_bass_kernel_spmd

NCORES = 8
B, S, E, H, R = 512, 50, 128, 8, 32
D = E // H
USERS, ITEMS, ALLSEQ, NCLASS = 50000, 20000, 20000, 5
BC = B // NCORES            # 64 batch rows per core
J = BC * S                  # 3200 (b, s) pairs per core
G = J // 128                # 25 gather groups of 128 pairs
KROW = 2 * R * E            # 8192 combined-table row (kiu | kui)
# b-aligned chunks (multiples of S); two small priming chunks let the
# attention->profile software pipeline fill faster
CH400 = [(0, 200), (200, 200)] + [(c * 400, 400) for c in range(1, J // 400)]
# K-gather units: one group per indirect gather (finer DMA granularity keeps
# the serial DMA bus fed and more groups in flight)
UNITS = [(g, 1) for g in range(G)]
# reduce mode per group: 'a' all-DVE chain, 'b' fold1-DMA + DVE rest,
# 'c' fold1-DMA + Pool rest, 'e' fold1-DVE + Pool rest
MODES = ['e', 'b', 'e', 'a', 'e', 'b', 'e', 'e', 'b', 'a', 'e', 'b', 'e',
         'e', 'b', 'a', 'e', 'b', 'e', 'e', 'b', 'a', 'e', 'c', 'c']
NB_CHUNKS = [(0, 9), (9, 8), (17, 8)]   # neighbor mega-gather splits
KPREFETCH = 6               # groups of K gather issued ahead of compute
KPRI = True                 # high-priority band for indirect gathers
APRI = True                 # high-priority band for attention

from contextlib import nullcontext as _nullctx

F32 = mybir.dt.float32
BF16 = mybir.dt.bfloat16
I32 = mybir.dt.int32
AF = mybir.ActivationFunctionType
ALU = mybir.AluOpType


def _ap(ap, dims):
    """Rebuild an AP with explicit [step, count] free dims (partition dim kept)."""
    return bass.AP(tensor=ap.tensor, offset=ap.offset, ap=[ap.ap[0]] + dims)


def _off(ap, extra_offset, dims):
    return bass.AP(tensor=ap.tensor, offset=ap.offset + extra_offset,
                   ap=[ap.ap[0]] + dims)


def build_nc(repeat=1):
    nc = bacc.Bacc(None, target_bir_lowering=False)

    p_user = nc.declare_dram_parameter("user_i", [BC, 1], I32, isOutput=False)
    p_nbg = nc.declare_dram_parameter("nbg_i", [128, G], I32, isOutput=False)
    p_seq = nc.declare_dram_parameter("seq_i", [128, G], I32, isOutput=False)
    p_itg = nc.declare_dram_parameter("itg_i", [128, G], I32, isOutput=False)
    p_negm = nc.declare_dram_parameter("negmask", [1, J], BF16, isOutput=False)
    p_uemb = nc.declare_dram_parameter("uemb", [USERS, E], F32, isOutput=False)
    p_item = nc.declare_dram_parameter("item_bf", [ITEMS, E], BF16, isOutput=False)
    p_kcat = nc.declare_dram_parameter("lcu_cat", [ALLSEQ, KROW], BF16, isOutput=False)
    p_wq = nc.declare_dram_parameter("wq", [E, E], F32, isOutput=False)
    p_wk = nc.declare_dram_parameter("wk", [E, E], BF16, isOutput=False)
    p_wv = nc.declare_dram_parameter("wv", [E, E], BF16, isOutput=False)
    p_wo = nc.declare_dram_parameter("wo", [E, E], F32, isOutput=False)
    p_bias = nc.declare_dram_parameter("biases", [E, 4], F32, isOutput=False)
    p_sel_eh = nc.declare_dram_parameter("sel_eh", [E, H], BF16, isOutput=False)
    p_sel_he = nc.declare_dram_parameter("sel_he", [H, E], BF16, isOutput=False)
    p_fcwb = nc.declare_dram_parameter("fc_wb", [R + 1, NCLASS], F32, isOutput=False)
    p_out = nc.declare_dram_parameter("out", [BC, NCLASS], F32, isOutput=True)
    p_ur = nc.declare_dram_parameter("ur_dbg", [R, BC], F32, isOutput=True)

    with tile.TileContext(nc) as tc:
        with (
            tc.tile_pool(name="const", bufs=1) as cpool,
            tc.tile_pool(name="big", bufs=1) as bpool,
            tc.tile_pool(name="work", bufs=3) as wpool,
            tc.tile_pool(name="kwork", bufs=6) as kpool,
            tc.tile_pool(name="ps_att", bufs=3, space="PSUM") as pp_att,
            tc.tile_pool(name="ps_lg", bufs=2, space="PSUM") as pp_lg,
            tc.tile_pool(name="ps_tp", bufs=2, space="PSUM") as pp_tp,
            tc.tile_pool(name="ps_l2", bufs=1, space="PSUM") as pp_l2,
        ):
            # ---------- constants (loaded once, outside the repeat loop) ----------
            t_ident = cpool.tile([128, 128], F32)
            make_identity(nc, t_ident[:])
            t_user = cpool.tile([BC, 1], I32)
            nc.sync.dma_start(out=t_user[:], in_=p_user[:])
            t_nbg = cpool.tile([128, G], I32)
            nc.sync.dma_start(out=t_nbg[:], in_=p_nbg[:])
            t_seq = cpool.tile([128, G], I32)
            nc.sync.dma_start(out=t_seq[:], in_=p_seq[:])
            t_itg = cpool.tile([128, G], I32)
            nc.sync.dma_start(out=t_itg[:], in_=p_itg[:])
            t_negm = cpool.tile([1, J], BF16)
            nc.sync.dma_start(out=t_negm[:], in_=p_negm[:])
            t_wq = cpool.tile([E, E], F32)
            nc.sync.dma_start(out=t_wq[:], in_=p_wq[:])
            t_wk = cpool.tile([E, E], BF16)
            nc.sync.dma_start(out=t_wk[:], in_=p_wk[:])
            t_wv = cpool.tile([E, E], BF16)
            nc.sync.dma_start(out=t_wv[:], in_=p_wv[:])
            t_wo = cpool.tile([E, E], F32)
            nc.sync.dma_start(out=t_wo[:], in_=p_wo[:])
            t_bias = cpool.tile([E, 4], F32)
            nc.sync.dma_start(out=t_bias[:], in_=p_bias[:])
            t_sel_eh = cpool.tile([E, H], BF16)
            nc.sync.dma_start(out=t_sel_eh[:], in_=p_sel_eh[:])
            t_sel_he = cpool.tile([H, E], BF16)
            nc.sync.dma_start(out=t_sel_he[:], in_=p_sel_he[:])
            t_fcwb = cpool.tile([R + 1, NCLASS], F32)
            nc.sync.dma_start(out=t_fcwb[:], in_=p_fcwb[:])
            t_ones18 = cpool.tile([1, H], BF16)
            nc.vector.memset(t_ones18[:], 1.0)
            # per-pair padding mask (1.0 where neighbor > 0), [128, G]
            t_wcol = cpool.tile([128, G], F32)
            nc.vector.tensor_scalar(out=t_wcol[:], in0=t_nbg[:], scalar1=0,
                                    scalar2=None, op0=ALU.is_gt)

            def body():
                # user rows -> [BC, E] -> transpose -> uT [E, BC]
                t_u = wpool.tile([BC, E], F32, tag="gath_u")
                nc.gpsimd.indirect_dma_start(
                    out=t_u[:], out_offset=None, in_=p_uemb[:],
                    in_offset=bass.IndirectOffsetOnAxis(ap=t_user[:, :1], axis=0))
                ps_uT = pp_tp.tile([E, BC], F32, tag="tp")
                nc.tensor.transpose(out=ps_uT[:], in_=t_u[:], identity=t_ident[:BC, :BC])
                t_uT = bpool.tile([E, BC], F32)
                nc.scalar.copy(out=t_uT[:], in_=ps_uT[:])

                # qT = (wq.T @ uT + bq) * (1/sqrt(D))  -- scale folded in here
                ps_q = pp_tp.tile([E, BC], F32, tag="tp")
                nc.tensor.matmul(out=ps_q[:], lhsT=t_wq[:], rhs=t_uT[:])
                t_qT = bpool.tile([E, BC], F32)
                nc.scalar.activation(out=t_qT[:], in_=ps_q[:], func=AF.Identity,
                                     bias=t_bias[:, 0:1], scale=1.0 / np.sqrt(D))

                # neighbor rows, mega-gathered in 3 chunks
                t_nball = bpool.tile([128, J], F32)
                t_nbT = bpool.tile([E, J], BF16)
                t_kT = bpool.tile([E, J], BF16)
                t_vT = bpool.tile([E, J], F32)
                t_att = bpool.tile([H, J], BF16)
                t_oT = bpool.tile([E, J], F32)
                t_ratT = bpool.tile([R, J], BF16)
                # multiplier tile: left half ctxo*w (pair-major), right half item
                t_mul = bpool.tile([128, 2 * J], BF16)
                with tc.high_priority(offset=100000) if KPRI else _nullctx():
                    nc.gpsimd.indirect_dma_start(
                        out=t_mul[:, J:2 * J], out_offset=None, in_=p_item[:],
                        in_offset=bass.IndirectOffsetOnAxis(ap=t_itg[:, 0:G], axis=0))

                def emit_nb_chunk(c0, cg):
                    with tc.high_priority(offset=100000) if KPRI else _nullctx():
                        nc.gpsimd.indirect_dma_start(
                            out=t_nball[:, c0 * 128:(c0 + cg) * 128], out_offset=None,
                            in_=p_uemb[:],
                            in_offset=bass.IndirectOffsetOnAxis(ap=t_nbg[:, c0:c0 + cg],
                                                                axis=0))

                def emit_nb_group(g):
                    # neighbor rows for group g, transposed into nbT slice
                    with tc.high_priority(offset=50000) if APRI else _nullctx():
                        ps_t = pp_tp.tile([128, 128], F32, tag="tp")
                        nc.tensor.transpose(out=ps_t[:],
                                            in_=t_nball[:, g * 128:(g + 1) * 128],
                                            identity=t_ident[:])
                        nc.scalar.copy(out=t_nbT[:, g * 128:(g + 1) * 128], in_=ps_t[:])

                def emit_attn_a(ci, c0, cn):
                    ctx_pri = tc.high_priority(offset=50000) if APRI else _nullctx()
                    ctx_pri.__enter__()
                    sl = slice(c0, c0 + cn)
                    nb = cn // S
                    b0 = c0 // S
                    # k/v projections for this chunk
                    ps_k = pp_att.tile([E, 400], F32, tag="att")
                    nc.tensor.matmul(out=ps_k[:, :cn], lhsT=t_wk[:],
                                     rhs=t_nbT[:, sl])
                    nc.scalar.activation(out=t_kT[:, sl], in_=ps_k[:, :cn],
                                         func=AF.Identity, bias=t_bias[:, 1:2], scale=1.0)
                    ps_v = pp_att.tile([E, 400], F32, tag="att")
                    nc.tensor.matmul(out=ps_v[:, :cn], lhsT=t_wv[:],
                                     rhs=t_nbT[:, sl])
                    nc.scalar.activation(out=t_vT[:, sl], in_=ps_v[:, :cn],
                                         func=AF.Identity, bias=t_bias[:, 2:3], scale=1.0)
                    # prod_qk = kT * q_b (per-b Act pass; 1/sqrt(D) already in qT)
                    for bi in range(nb):
                        bsl = slice(c0 + bi * S, c0 + (bi + 1) * S)
                        nc.scalar.activation(out=t_kT[:, bsl], in_=t_kT[:, bsl],
                                             func=AF.Identity,
                                             scale=t_qT[:, b0 + bi:b0 + bi + 1])
                    # logits = per-head sums + negmask; attn = exp(logits)
                    # (masked entries underflow to exactly 0, matching the
                    # reference softmax); normalized by the row sum below.
                    ps_lg = pp_lg.tile([H, 400], F32, tag="lg")
                    nc.tensor.matmul(out=ps_lg[:, :cn], lhsT=t_sel_eh[:], rhs=t_kT[:, sl],
                                     start=True, stop=False)
                    nc.tensor.matmul(out=ps_lg[:, :cn], lhsT=t_ones18[:], rhs=t_negm[:, sl],
                                     start=False, stop=True)
                    t_ssc = wpool.tile([H, 8], F32, tag="ssc")
                    ss_tiles[ci] = t_ssc
                    for bi in range(cn // S):
                        nc.scalar.activation(out=t_att[:, c0 + bi * S:c0 + (bi + 1) * S],
                                             in_=ps_lg[:, bi * S:(bi + 1) * S], func=AF.Exp,
                                             accum_out=t_ssc[:, bi:bi + 1])
                    ctx_pri.__exit__(None, None, None)

                def emit_attn_b(ci, c0, cn):
                    ctx_pri = tc.high_priority(offset=50000) if APRI else _nullctx()
                    ctx_pri.__enter__()
                    sl = slice(c0, c0 + cn)
                    nb = cn // S
                    t_ssc = ss_tiles.pop(ci)
                    t_rs = wpool.tile([H, 8], F32, tag="sm2")
                    nc.vector.reciprocal(out=t_rs[:, :nb], in_=t_ssc[:, :nb])
                    for bi in range(nb):
                        bsl = slice(c0 + bi * S, c0 + (bi + 1) * S)
                        nc.scalar.activation(out=t_att[:, bsl], in_=t_att[:, bsl],
                                             func=AF.Identity,
                                             scale=t_rs[:, bi:bi + 1])
                    # ctxT = attn_bcast * vT (in place over vT); ctxo = wo.T@ctx + bo
                    ps_ab = pp_att.tile([E, 400], F32, tag="att")
                    nc.tensor.matmul(out=ps_ab[:, :cn], lhsT=t_sel_he[:], rhs=t_att[:, sl])
                    nc.vector.tensor_tensor(out=t_vT[:, sl], in0=t_vT[:, sl],
                                            in1=ps_ab[:, :cn], op=ALU.mult)
                    ps_o = pp_att.tile([E, 400], F32, tag="att")
                    nc.tensor.matmul(out=ps_o[:, :cn], lhsT=t_wo[:], rhs=t_vT[:, sl])
                    nc.scalar.activation(out=t_oT[:, sl], in_=ps_o[:, :cn],
                                         func=AF.Identity, bias=t_bias[:, 3:4], scale=1.0)
                    ctx_pri.__exit__(None, None, None)

                k_tiles = {}
                ss_tiles = {}

                def emit_k_gather(g):
                    t_k = kpool.tile([128, KROW], BF16, tag="k")
                    k_tiles[g] = t_k
                    with tc.high_priority(offset=100000) if KPRI else _nullctx():
                        nc.gpsimd.indirect_dma_start(
                            out=t_k[:], out_offset=None, in_=p_kcat[:],
                            in_offset=bass.IndirectOffsetOnAxis(ap=t_seq[:, g:g + 1],
                                                                axis=0))

                def emit_mul_prep(g):
                    # ctxo (pair-major, scaled by the padding mask) -> mul left half
                    with tc.high_priority(offset=40000) if APRI else _nullctx():
                        ps_tp = pp_tp.tile([128, 128], F32, tag="tp")
                        nc.tensor.transpose(out=ps_tp[:],
                                            in_=t_oT[:, g * 128:(g + 1) * 128],
                                            identity=t_ident[:])
                        nc.scalar.activation(out=t_mul[:, g * 128:(g + 1) * 128],
                                             in_=ps_tp[:], func=AF.Identity,
                                             scale=t_wcol[:, g:g + 1])

                def emit_product(g):
                    # product: K[t,m,r,e] *= mul[(t),m-permuted e] broadcast
                    # over r (mul halves: t=0 ctxo' at cols g*128.., t=1 item
                    # at J + g*128..), then fold1 (m=0 half += m=1 half;
                    # contiguous 2048-elem chunks so the DMA path is cheap)
                    t_k = k_tiles[g]
                    k_in = _ap(t_k[:],
                               [[R * E, 2], [R * E // 2, 2], [E // 2, R],
                                [1, E // 2]])
                    mul_in = _off(t_mul[:], g * 128,
                                  [[J, 2], [E // 2, 2], [0, R], [1, E // 2]])
                    nc.vector.tensor_tensor(out=k_in, in0=k_in, in1=mul_in,
                                            op=ALU.mult)
                    f1_out = _ap(t_k[:], [[R * E, 2], [1, R * E // 2]])
                    f1_in = _off(t_k[:], R * E // 2, [[R * E, 2], [1, R * E // 2]])
                    if MODES[g] in ('b', 'c'):
                        nc.gpsimd.dma_start(out=f1_out, in_=f1_in, accum_op=ALU.add)
                    else:
                        nc.vector.tensor_tensor(out=f1_out, in0=f1_out, in1=f1_in,
                                                op=ALU.add)

                def emit_chain(g):
                    # reduce over e via halving folds on the mode's engine;
                    # layout per table chunk is (r, e_low) after fold1
                    t_k = k_tiles[g]
                    km = lambda h: _ap(t_k[:], [[R * E, 2], [E // 2, R], [1, h]])
                    kmh = lambda h: _off(t_k[:], h,
                                         [[R * E, 2], [E // 2, R], [1, h]])
                    eng = nc.gpsimd if MODES[g] in ('c', 'e') else nc.vector
                    w = E // 2
                    while w > 1:
                        h = w // 2
                        eng.tensor_tensor(out=km(h), in0=km(h), in1=kmh(h),
                                          op=ALU.add)
                        w = h

                def emit_rating(g):
                    # rating = nprof' * iprof (w already folded into ctxo side)
                    t_k = k_tiles.pop(g)
                    t_rat = wpool.tile([128, R], F32, tag="rat")
                    rat_n = _ap(t_k[:], [[E // 2, R]])
                    rat_i = _off(t_k[:], R * E, [[E // 2, R]])
                    nc.vector.tensor_tensor(out=t_rat[:], in0=rat_n, in1=rat_i,
                                            op=ALU.mult)
                    ps_rt = pp_tp.tile([R, 128], F32, tag="tp")
                    nc.tensor.transpose(out=ps_rt[:], in_=t_rat[:],
                                        identity=t_ident[:])
                    nc.scalar.copy(out=t_ratT[:, g * 128:(g + 1) * 128],
                                   in_=ps_rt[:])

                # modulo-software-pipelined emission. Per-engine instruction
                # streams execute essentially in emission order (head-of-line
                # at each engine queue), so each group's reduce stages are
                # emitted lagged behind later groups' earlier stages: by the
                # time an engine reaches chain(g), fold1(g) has long finished.
                nbc = 0
                g_nb = 0
                g_kg = 0
                g_prep = 0
                s_prod = 0
                s_chain = 0
                s_rat = 0

                def drain_pipe(prod_hi, chain_lag, rat_lag):
                    nonlocal s_prod, s_chain, s_rat
                    while s_prod < prod_hi:
                        emit_product(s_prod)
                        s_prod += 1
                        while s_chain < s_prod - chain_lag:
                            emit_chain(s_chain)
                            s_chain += 1
                        while s_rat < s_chain - rat_lag:
                            emit_rating(s_rat)
                            s_rat += 1

                for ci, (c0, cn) in enumerate(CH400):
                    hi = c0 + cn
                    while nbc < len(NB_CHUNKS) and NB_CHUNKS[nbc][0] * 128 < hi:
                        emit_nb_chunk(*NB_CHUNKS[nbc])
                        nbc += 1
                    while g_nb * 128 < hi:
                        emit_nb_group(g_nb)
                        g_nb += 1
                    while g_kg < G and g_kg * 128 <= hi + KPREFETCH * 128:
                        emit_k_gather(g_kg)
                        g_kg += 1
                    emit_attn_a(ci, c0, cn)
                    drain_pipe(max(0, g_prep - 1), 2, 1)
                    emit_attn_b(ci, c0, cn)
                    while (g_prep + 1) * 128 <= hi:
                        emit_mul_prep(g_prep)
                        g_prep += 1
                while g_kg < G:
                    emit_k_gather(g_kg)
                    g_kg += 1
                while g_prep < G:
                    emit_mul_prep(g_prep)
                    g_prep += 1
                drain_pipe(G, 2, 1)
                while s_chain < G:
                    emit_chain(s_chain)
                    s_chain += 1
                while s_rat < G:
                    emit_rating(s_rat)
                    s_rat += 1

                # user rating vector: max over s
                t_urp = wpool.tile([R + 1, BC], F32, tag="urp")
                nc.vector.tensor_reduce(out=t_urp[:R, :],
                                        in_=t_ratT[:].rearrange("r (b s) -> r b s", s=S),
                                        axis=mybir.AxisListType.X, op=ALU.max)
                nc.vector.memset(t_urp[R:R + 1, :], 1.0)
                nc.sync.dma_start(out=p_ur[:], in_=t_urp[:R, :])

                # classifier + softmax
                ps_l2 = pp_l2.tile([BC, NCLASS], F32, tag="l2")
                nc.tensor.matmul(out=ps_l2[:], lhsT=t_urp[:], rhs=t_fcwb[:])
                t_nm2 = wpool.tile([BC, 1], F32, tag="fin")
                nc.vector.tensor_reduce(out=t_nm2[:], in_=ps_l2[:],
                                        axis=mybir.AxisListType.X,
                                        op=ALU.max, negate=True)
                t_e2 = wpool.tile([BC, NCLASS], F32, tag="fin2")
                t_s2 = wpool.tile([BC, 1], F32, tag="fin3")
                nc.scalar.activation(out=t_e2[:], in_=ps_l2[:], func=AF.Exp,
                                     bias=t_nm2[:, :1], scale=1.0,
                                     accum_out=t_s2[:, :1])
                t_r2 = wpool.tile([BC, 1], F32, tag="fin4")
                nc.vector.reciprocal(out=t_r2[:], in_=t_s2[:])
                t_o = wpool.tile([BC, NCLASS], F32, tag="fin5")
                nc.vector.tensor_scalar(out=t_o[:], in0=t_e2[:], scalar1=t_r2[:, :1],
                                        scalar2=None, op0=ALU.mult)
                nc.sync.dma_start(out=p_out[:], in_=t_o[:])

            if repeat == 1:
                body()
            else:
                with tc.For_i(0, repeat, 1):
                    body()

    nc.finalize()
    return nc


def prep_in_maps(inputs):
    user = np.asarray(inputs["user"]).astype(np.int32).reshape(B)
    item = np.asarray(inputs["item"]).astype(np.int32).reshape(B)
    neighbor = np.asarray(inputs["neighbor"]).astype(np.int32).reshape(B, S)
    seq = np.asarray(inputs["seq"]).astype(np.int32).reshape(B, S)

    f32 = lambda x: np.ascontiguousarray(np.asarray(x, dtype=np.float32))
    bf16 = lambda x: np.ascontiguousarray(
        np.asarray(x, dtype=np.float32).astype(ml_dtypes.bfloat16))

    uemb = f32(inputs["user_emb_table"])
    item_bf = bf16(inputs["item_emb_table"])
    # both LCU tables in one row: (t, e_msb, r, e_low) so the first fold's
    # halves are contiguous 2048-elem chunks per table
    perm = lambda t: t.reshape(ALLSEQ, R, 2, E // 2).transpose(0, 2, 1, 3).reshape(
        ALLSEQ, R * E)
    lcu_cat = np.ascontiguousarray(np.concatenate(
        [perm(bf16(inputs["item_user_LCU"])),
         perm(bf16(inputs["user_item_LCU"]))], axis=1))
    biases = np.ascontiguousarray(np.stack(
        [f32(inputs["bq"]), f32(inputs["bk"]),
         f32(inputs["bv"]), f32(inputs["bo"])], axis=1))
    sel_eh = np.zeros((E, H), np.float32)
    sel_eh[np.arange(E), np.arange(E) // D] = 1.0
    fc_wb = np.ascontiguousarray(np.concatenate(
        [f32(inputs["fc_w"]), f32(inputs["fc_b"]).reshape(1, NCLASS)], axis=0))

    shared = {
        "uemb": uemb, "item_bf": item_bf, "lcu_cat": lcu_cat,
        "wq": f32(inputs["wq"]), "wk": bf16(inputs["wk"]),
        "wv": bf16(inputs["wv"]), "wo": f32(inputs["wo"]),
        "biases": biases, "sel_eh": np.ascontiguousarray(sel_eh.astype(ml_dtypes.bfloat16)),
        "sel_he": np.ascontiguousarray(sel_eh.T.astype(ml_dtypes.bfloat16)), "fc_wb": fc_wb,
    }
    in_maps = []
    for c in range(NCORES):
        bsl = slice(c * BC, (c + 1) * BC)
        nb = neighbor[bsl].reshape(J)
        sq = seq[bsl].reshape(J)
        itx = np.repeat(item[bsl], S)
        col = lambda x: np.ascontiguousarray(x.reshape(G, 128).T.astype(np.int32))
        in_maps.append({
            **shared,
            "user_i": np.ascontiguousarray(user[bsl].reshape(BC, 1)),
            "nbg_i": col(nb), "seq_i": col(sq), "itg_i": col(itx),
            "negmask": np.ascontiguousarray(
                (-1e9 * (nb <= 0)).astype(ml_dtypes.bfloat16).reshape(1, J)),
        })
    return in_maps


_NC_CACHE = {}


def kernel(**inputs):
    if "nc" not in _NC_CACHE:
        _NC_CACHE["nc"] = build_nc(repeat=1)
    nc = _NC_CACHE["nc"]
    in_maps = prep_in_maps(inputs)
    res = run_bass_kernel_spmd(nc, in_maps, core_ids=list(range(NCORES)))
    return np.concatenate([res.results[c]["out"] for c in range(NCORES)], axis=0)


# revision 25
# speedup vs baseline: 1.1594x; 1.0122x over previous
"""ARERec forward kernel for 8 TRN2 NeuronCores.

Data-parallel over batch: each core processes B/8 = 64 batch rows end-to-end
(embedding gathers, single-query multi-head attention, LCU region profiles,
rating classifier); tables and weights are replicated. The final [512, 5]
softmax output is concatenated on the host from the 8 per-core [64, 5] shards.

Engine balance (per core, cost-model ns): the serial DMA bus moves ~52MB of
LCU rows (~150us floor); DVE holds the region-profile products; the fold
chains that reduce each product over e are split between DVE, DMA-accumulate
(CCE add) and Pool (gpsimd) so no single engine exceeds the DMA floor by
much. Both LCU tables are host-merged into one [ALLSEQ, 2*R*E] row so each
128-pair group needs one gather; item/neighbor rows use one mega-gather each.

Self-contained: shapes/sharding are hardcoded from the problem spec.
"""
import numpy as np
import ml_dtypes

import concourse.bacc as bacc
import concourse.bass as bass
import concourse.mybir as mybir
import concourse.tile as tile
from concourse.masks import make_identity
from concourse.bass_utils import run_bass_kernel_spmd

NCORES = 8
B, S, E, H, R = 512, 50, 128, 8, 32
D = E // H
USERS, ITEMS, ALLSEQ, NCLASS = 50000, 20000, 20000, 5
BC = B // NCORES            # 64 batch rows per core
J = BC * S                  # 3200 (b, s) pairs per core
G = J // 128                # 25 gather groups of 128 pairs
KROW = 2 * R * E            # 8192 combined-table row (kiu | kui)
# b-aligned chunks (multiples of S); two small priming chunks let the
# attention->profile software pipeline fill faster
CH400 = [(0, 200), (200, 200)] + [(c * 400, 400) for c in range(1, J // 400)]
# K-gather units: one group per indirect gather (finer DMA granularity keeps
# the serial DMA bus fed and more groups in flight)
UNITS = [(g, 1) for g in range(G)]
# reduce mode per group: 'a' all-DVE chain, 'b' fold1-DMA + DVE rest,
# 'c' fold1-DMA + Pool rest, 'e' fold1-DVE + Pool rest
MODES = ['e', 'b', 'e', 'a', 'e', 'b', 'e', 'e', 'b', 'a', 'e', 'b', 'e',
         'e', 'b', 'a', 'e', 'b', 'e', 'e', 'b', 'a', 'e', 'c', 'c']
NB_CHUNKS = [(0, 9), (9, 8), (17, 8)]   # neighbor mega-gather splits
KPREFETCH = 6               # groups of K gather issued ahead of compute
KPRI = True                 # high-priority band for indirect gathers
APRI = True                 # high-priority band for attention
PROFILES = True             # emit the profile (K) pipeline; False = attention only

from contextlib import nullcontext as _nullctx

F32 = mybir.dt.float32
BF16 = mybir.dt.bfloat16
I32 = mybir.dt.int32
AF = mybir.ActivationFunctionType
ALU = mybir.AluOpType


def _ap(ap, dims):
    """Rebuild an AP with explicit [step, count] free dims (partition dim kept)."""
    return bass.AP(tensor=ap.tensor, offset=ap.offset, ap=[ap.ap[0]] + dims)


def _off(ap, extra_offset, dims):
    return bass.AP(tensor=ap.tensor, offset=ap.offset + extra_offset,
                   ap=[ap.ap[0]] + dims)


def build_nc(repeat=1):
    nc = bacc.Bacc(None, target_bir_lowering=False)

    p_user = nc.declare_dram_parameter("user_i", [BC, 1], I32, isOutput=False)
    p_nbg = nc.declare_dram_parameter("nbg_i", [128, G], I32, isOutput=False)
    p_seq = nc.declare_dram_parameter("seq_i", [128, G], I32, isOutput=False)
    p_itg = nc.declare_dram_parameter("itg_i", [128, G], I32, isOutput=False)
    p_negm = nc.declare_dram_parameter("negmask", [1, J], BF16, isOutput=False)
    p_uemb = nc.declare_dram_parameter("uemb", [USERS, E], F32, isOutput=False)
    p_item = nc.declare_dram_parameter("item_bf", [ITEMS, E], BF16, isOutput=False)
    p_kcat = nc.declare_dram_parameter("lcu_cat", [ALLSEQ, KROW], BF16, isOutput=False)
    p_wq = nc.declare_dram_parameter("wq", [E, E], F32, isOutput=False)
    p_wk = nc.declare_dram_parameter("wk", [E, E], BF16, isOutput=False)
    p_wv = nc.declare_dram_parameter("wv", [E, E], BF16, isOutput=False)
    p_wo = nc.declare_dram_parameter("wo", [E, E], F32, isOutput=False)
    p_bias = nc.declare_dram_parameter("biases", [E, 4], F32, isOutput=False)
    p_sel_eh = nc.declare_dram_parameter("sel_eh", [E, H], BF16, isOutput=False)
    p_sel_he = nc.declare_dram_parameter("sel_he", [H, E], BF16, isOutput=False)
    p_fcwb = nc.declare_dram_parameter("fc_wb", [R + 1, NCLASS], F32, isOutput=False)
    p_out = nc.declare_dram_parameter("out", [BC, NCLASS], F32, isOutput=True)
    p_ur = nc.declare_dram_parameter("ur_dbg", [R, BC], F32, isOutput=True)

    with tile.TileContext(nc) as tc:
        with (
            tc.tile_pool(name="const", bufs=1) as cpool,
            tc.tile_pool(name="big", bufs=1) as bpool,
            tc.tile_pool(name="work", bufs=3) as wpool,
            tc.tile_pool(name="kwork", bufs=6) as kpool,
            tc.tile_pool(name="ps_att", bufs=4, space="PSUM") as pp_att,
            tc.tile_pool(name="ps_lg", bufs=2, space="PSUM") as pp_lg,
            tc.tile_pool(name="ps_tp", bufs=2, space="PSUM") as pp_tp,
        ):
            # ---------- constants (loaded once, outside the repeat loop) ----------
            t_ident = cpool.tile([128, 128], F32)
            make_identity(nc, t_ident[:])
            t_user = cpool.tile([BC, 1], I32)
            nc.sync.dma_start(out=t_user[:], in_=p_user[:])
            t_nbg = cpool.tile([128, G], I32)
            nc.sync.dma_start(out=t_nbg[:], in_=p_nbg[:])
            t_seq = cpool.tile([128, G], I32)
            nc.sync.dma_start(out=t_seq[:], in_=p_seq[:])
            t_itg = cpool.tile([128, G], I32)
            nc.sync.dma_start(out=t_itg[:], in_=p_itg[:])
            t_negm = cpool.tile([1, J], BF16)
            nc.sync.dma_start(out=t_negm[:], in_=p_negm[:])
            t_wq = cpool.tile([E, E], F32)
            nc.sync.dma_start(out=t_wq[:], in_=p_wq[:])
            t_wk = cpool.tile([E, E], BF16)
            nc.sync.dma_start(out=t_wk[:], in_=p_wk[:])
            t_wv = cpool.tile([E, E], BF16)
            nc.sync.dma_start(out=t_wv[:], in_=p_wv[:])
            t_wo = cpool.tile([E, E], F32)
            nc.sync.dma_start(out=t_wo[:], in_=p_wo[:])
            t_bias = cpool.tile([E, 4], F32)
            nc.sync.dma_start(out=t_bias[:], in_=p_bias[:])
            t_sel_eh = cpool.tile([E, H], BF16)
            nc.sync.dma_start(out=t_sel_eh[:], in_=p_sel_eh[:])
            t_sel_he = cpool.tile([H, E], BF16)
            nc.sync.dma_start(out=t_sel_he[:], in_=p_sel_he[:])
            t_fcwb = cpool.tile([R + 1, NCLASS], F32)
            nc.sync.dma_start(out=t_fcwb[:], in_=p_fcwb[:])
            t_ones18 = cpool.tile([1, H], BF16)
            nc.vector.memset(t_ones18[:], 1.0)
            # per-pair padding mask (1.0 where neighbor > 0), [128, G]
            t_wcol = cpool.tile([128, G], F32)
            nc.vector.tensor_scalar(out=t_wcol[:], in0=t_nbg[:], scalar1=0,
                                    scalar2=None, op0=ALU.is_gt)

            def body():
                # user rows -> [BC, E] -> transpose -> uT [E, BC]
                t_u = wpool.tile([BC, E], F32, tag="gath_u")
                nc.gpsimd.indirect_dma_start(
                    out=t_u[:], out_offset=None, in_=p_uemb[:],
                    in_offset=bass.IndirectOffsetOnAxis(ap=t_user[:, :1], axis=0))
                ps_uT = pp_tp.tile([E, BC], F32, tag="tp")
                nc.tensor.transpose(out=ps_uT[:], in_=t_u[:], identity=t_ident[:BC, :BC])
                t_uT = bpool.tile([E, BC], F32)
                nc.scalar.copy(out=t_uT[:], in_=ps_uT[:])

                # qT = (wq.T @ uT + bq) * (1/sqrt(D))  -- scale folded in here
                ps_q = pp_tp.tile([E, BC], F32, tag="tp")
                nc.tensor.matmul(out=ps_q[:], lhsT=t_wq[:], rhs=t_uT[:])
                t_qT = bpool.tile([E, BC], F32)
                nc.scalar.activation(out=t_qT[:], in_=ps_q[:], func=AF.Identity,
                                     bias=t_bias[:, 0:1], scale=1.0 / np.sqrt(D))
                # bk folded into the fused k-copy+qk pass: bias = bk * q_b
                t_bq2 = bpool.tile([E, BC], F32)
                nc.vector.tensor_scalar(out=t_bq2[:], in0=t_qT[:],
                                        scalar1=t_bias[:, 1:2], scalar2=None,
                                        op0=ALU.mult)

                # neighbor rows, mega-gathered in 3 chunks
                t_nball = bpool.tile([128, J], F32)
                t_nbT = bpool.tile([E, J], BF16)
                t_kT = bpool.tile([E, J], BF16)
                t_vT = bpool.tile([E, J], F32)
                t_att = bpool.tile([H, J], BF16)
                t_oT = bpool.tile([E, J], F32)
                t_ratT = bpool.tile([R, J], BF16)
                if not PROFILES:
                    nc.vector.memset(t_ratT[:], 0.0)
                # multiplier tile: left half ctxo*w (pair-major), right half item
                t_mul = bpool.tile([128, 2 * J], BF16)
                with tc.high_priority(offset=100000) if KPRI else _nullctx():
                    nc.gpsimd.indirect_dma_start(
                        out=t_mul[:, J:2 * J], out_offset=None, in_=p_item[:],
                        in_offset=bass.IndirectOffsetOnAxis(ap=t_itg[:, 0:G], axis=0))

                def emit_nb_chunk(c0, cg):
                    with tc.high_priority(offset=100000) if KPRI else _nullctx():
                        nc.gpsimd.indirect_dma_start(
                            out=t_nball[:, c0 * 128:(c0 + cg) * 128], out_offset=None,
                            in_=p_uemb[:],
                            in_offset=bass.IndirectOffsetOnAxis(ap=t_nbg[:, c0:c0 + cg],
                                                                axis=0))

                def emit_nb_group(g):
                    # neighbor rows for group g, transposed into nbT slice
                    with tc.high_priority(offset=50000) if APRI else _nullctx():
                        ps_t = pp_tp.tile([128, 128], F32, tag="tp")
                        nc.tensor.transpose(out=ps_t[:],
                                            in_=t_nball[:, g * 128:(g + 1) * 128],
                                            identity=t_ident[:])
                        nc.scalar.copy(out=t_nbT[:, g * 128:(g + 1) * 128], in_=ps_t[:])

                def emit_attn_a(ci, c0, cn):
                    ctx_pri = tc.high_priority(offset=50000) if APRI else _nullctx()
                    ctx_pri.__enter__()
                    sl = slice(c0, c0 + cn)
                    nb = cn // S
                    b0 = c0 // S
                    # k/v projections for this chunk
                    ps_k = pp_att.tile([E, 400], F32, tag="att")
                    nc.tensor.matmul(out=ps_k[:, :cn], lhsT=t_wk[:],
                                     rhs=t_nbT[:, sl])
                    ps_v = pp_att.tile([E, 400], F32, tag="att")
                    nc.tensor.matmul(out=ps_v[:, :cn], lhsT=t_wv[:],
                                     rhs=t_nbT[:, sl])
                    nc.scalar.activation(out=t_vT[:, sl], in_=ps_v[:, :cn],
                                         func=AF.Identity, bias=t_bias[:, 2:3], scale=1.0)
                    # fused copy: kT = (k_proj + bk) * q_b = k_proj*q_b + bq2_b
                    for bi in range(nb):
                        bsl = slice(c0 + bi * S, c0 + (bi + 1) * S)
                        nc.scalar.activation(out=t_kT[:, bsl],
                                             in_=ps_k[:, bi * S:(bi + 1) * S],
                                             func=AF.Identity,
                                             scale=t_qT[:, b0 + bi:b0 + bi + 1],
                                             bias=t_bq2[:, b0 + bi:b0 + bi + 1])
                    # logits = per-head sums + negmask; attn = exp(logits)
                    # (masked entries underflow to exactly 0, matching the
                    # reference softmax); normalized by the row sum below.
                    ps_lg = pp_lg.tile([H, 400], F32, tag="lg")
                    nc.tensor.matmul(out=ps_lg[:, :cn], lhsT=t_sel_eh[:], rhs=t_kT[:, sl],
                                     start=True, stop=False)
                    nc.tensor.matmul(out=ps_lg[:, :cn], lhsT=t_ones18[:], rhs=t_negm[:, sl],
                                     start=False, stop=True)
                    t_ssc = wpool.tile([H, 8], F32, tag="ssc")
                    ss_tiles[ci] = t_ssc
                    for bi in range(cn // S):
                        nc.scalar.activation(out=t_att[:, c0 + bi * S:c0 + (bi + 1) * S],
                                             in_=ps_lg[:, bi * S:(bi + 1) * S], func=AF.Exp,
                                             accum_out=t_ssc[:, bi:bi + 1])
                    ctx_pri.__exit__(None, None, None)

                def emit_attn_b(ci, c0, cn):
                    ctx_pri = tc.high_priority(offset=50000) if APRI else _nullctx()
                    ctx_pri.__enter__()
                    sl = slice(c0, c0 + cn)
                    nb = cn // S
                    t_ssc = ss_tiles.pop(ci)
                    t_rs = wpool.tile([H, 8], F32, tag="sm2")
                    nc.vector.reciprocal(out=t_rs[:, :nb], in_=t_ssc[:, :nb])
                    for bi in range(nb):
                        bsl = slice(c0 + bi * S, c0 + (bi + 1) * S)
                        nc.scalar.activation(out=t_att[:, bsl], in_=t_att[:, bsl],
                                             func=AF.Identity,
                                             scale=t_rs[:, bi:bi + 1])
                    # ctxT = attn_bcast * vT (in place over vT); ctxo = wo.T@ctx + bo
                    ps_ab = pp_att.tile([E, 400], F32, tag="att")
                    nc.tensor.matmul(out=ps_ab[:, :cn], lhsT=t_sel_he[:], rhs=t_att[:, sl])
                    nc.vector.tensor_tensor(out=t_vT[:, sl], in0=t_vT[:, sl],
                                            in1=ps_ab[:, :cn], op=ALU.mult)
                    ps_o = pp_att.tile([E, 400], F32, tag="att")
                    nc.tensor.matmul(out=ps_o[:, :cn], lhsT=t_wo[:], rhs=t_vT[:, sl])
                    nc.scalar.activation(out=t_oT[:, sl], in_=ps_o[:, :cn],
                                         func=AF.Identity, bias=t_bias[:, 3:4], scale=1.0)
                    ctx_pri.__exit__(None, None, None)

                k_tiles = {}
                ss_tiles = {}

                def emit_k_gather(g):
                    t_k = kpool.tile([128, KROW], BF16, tag="k")
                    k_tiles[g] = t_k
                    with tc.high_priority(offset=100000) if KPRI else _nullctx():
                        nc.gpsimd.indirect_dma_start(
                            out=t_k[:], out_offset=None, in_=p_kcat[:],
                            in_offset=bass.IndirectOffsetOnAxis(ap=t_seq[:, g:g + 1],
                                                                axis=0))

                def emit_mul_prep(g):
                    # ctxo (pair-major, scaled by the padding mask) -> mul left half
                    with tc.high_priority(offset=40000) if APRI else _nullctx():
                        ps_tp = pp_tp.tile([128, 128], F32, tag="tp")
                        nc.tensor.transpose(out=ps_tp[:],
                                            in_=t_oT[:, g * 128:(g + 1) * 128],
                                            identity=t_ident[:])
                        nc.scalar.activation(out=t_mul[:, g * 128:(g + 1) * 128],
                                             in_=ps_tp[:], func=AF.Identity,
                                             scale=t_wcol[:, g:g + 1])

                def emit_product(g):
                    # product: K[t,m,r,e] *= mul[(t),m-permuted e] broadcast
                    # over r (mul halves: t=0 ctxo' at cols g*128.., t=1 item
                    # at J + g*128..), then fold1 (m=0 half += m=1 half;
                    # contiguous 2048-elem chunks so the DMA path is cheap)
                    t_k = k_tiles[g]
                    k_in = _ap(t_k[:],
                               [[R * E, 2], [R * E // 2, 2], [E // 2, R],
                                [1, E // 2]])
                    mul_in = _off(t_mul[:], g * 128,
                                  [[J, 2], [E // 2, 2], [0, R], [1, E // 2]])
                    nc.vector.tensor_tensor(out=k_in, in0=k_in, in1=mul_in,
                                            op=ALU.mult)
                    f1_out = _ap(t_k[:], [[R * E, 2], [1, R * E // 2]])
                    f1_in = _off(t_k[:], R * E // 2, [[R * E, 2], [1, R * E // 2]])
                    if MODES[g] in ('b', 'c'):
                        nc.gpsimd.dma_start(out=f1_out, in_=f1_in, accum_op=ALU.add)
                    else:
                        nc.vector.tensor_tensor(out=f1_out, in0=f1_out, in1=f1_in,
                                                op=ALU.add)

                def emit_chain(g):
                    # reduce over e via halving folds on the mode's engine;
                    # layout per table chunk is (r, e_low) after fold1
                    t_k = k_tiles[g]
                    km = lambda h: _ap(t_k[:], [[R * E, 2], [E // 2, R], [1, h]])
                    kmh = lambda h: _off(t_k[:], h,
                                         [[R * E, 2], [E // 2, R], [1, h]])
                    eng = nc.gpsimd if MODES[g] in ('c', 'e') else nc.vector
                    w = E // 2
                    while w > 1:
                        h = w // 2
                        eng.tensor_tensor(out=km(h), in0=km(h), in1=kmh(h),
                                          op=ALU.add)
                        w = h

                def emit_rating(g):
                    # rating = nprof' * iprof (w already folded into ctxo side)
                    t_k = k_tiles.pop(g)
                    t_rat = wpool.tile([128, R], F32, tag="rat")
                    rat_n = _ap(t_k[:], [[E // 2, R]])
                    rat_i = _off(t_k[:], R * E, [[E // 2, R]])
                    nc.vector.tensor_tensor(out=t_rat[:], in0=rat_n, in1=rat_i,
                                            op=ALU.mult)
                    ps_rt = pp_tp.tile([R, 128], F32, tag="tp")
                    nc.tensor.transpose(out=ps_rt[:], in_=t_rat[:],
                                        identity=t_ident[:])
                    nc.scalar.copy(out=t_ratT[:, g * 128:(g + 1) * 128],
                                   in_=ps_rt[:])

                # modulo-software-pipelined emission. Per-engine instruction
                # streams execute essentially in emission order (head-of-line
                # at each engine queue), so each group's reduce stages are
                # emitted lagged behind later groups' earlier stages: by the
                # time an engine reaches chain(g), fold1(g) has long finished.
                nbc = 0
                g_nb = 0
                g_kg = 0
                g_prep = 0
                s_prod = 0
                s_chain = 0
                s_rat = 0

                def drain_pipe(prod_hi, chain_lag, rat_lag):
                    nonlocal s_prod, s_chain, s_rat
                    while s_prod < prod_hi:
                        emit_product(s_prod)
                        s_prod += 1
                        while s_chain < s_prod - chain_lag:
                            emit_chain(s_chain)
                            s_chain += 1
                        while s_rat < s_chain - rat_lag:
                            emit_rating(s_rat)
                            s_rat += 1

                for ci, (c0, cn) in enumerate(CH400):
                    hi = c0 + cn
                    while nbc < len(NB_CHUNKS) and NB_CHUNKS[nbc][0] * 128 < hi:
                        emit_nb_chunk(*NB_CHUNKS[nbc])
                        nbc += 1
                    while g_nb * 128 < hi:
                        emit_nb_group(g_nb)
                        g_nb += 1
                    while PROFILES and g_kg < G and g_kg * 128 <= hi + KPREFETCH * 128:
                        emit_k_gather(g_kg)
                        g_kg += 1
                    emit_attn_a(ci, c0, cn)
                    if PROFILES:
                        drain_pipe(max(0, g_prep - 1), 2, 1)
                    if ci > 0:
                        pc0, pcn = CH400[ci - 1]
                        emit_attn_b(ci - 1, pc0, pcn)
                        while (g_prep + 1) * 128 <= pc0 + pcn:
                            emit_mul_prep(g_prep)
                            g_prep += 1
                emit_attn_b(len(CH400) - 1, *CH400[-1])
                while g_prep < G:
                    emit_mul_prep(g_prep)
                    g_prep += 1
                if PROFILES:
                    while g_kg < G:
                        emit_k_gather(g_kg)
                        g_kg += 1
                    drain_pipe(G, 2, 1)
                    while s_chain < G:
                        emit_chain(s_chain)
                        s_chain += 1
                    while s_rat < G:
                        emit_rating(s_rat)
                        s_rat += 1

                # user rating vector: max over s
                t_urp = wpool.tile([R + 1, BC], F32, tag="urp")
                nc.vector.tensor_reduce(out=t_urp[:R, :],
                                        in_=t_ratT[:].rearrange("r (b s) -> r b s", s=S),
                                        axis=mybir.AxisListType.X, op=ALU.max)
                nc.vector.memset(t_urp[R:R + 1, :], 1.0)
                nc.sync.dma_start(out=p_ur[:], in_=t_urp[:R, :])

                # classifier + softmax
                ps_l2 = pp_lg.tile([BC, NCLASS], F32, tag="lg")
                nc.tensor.matmul(out=ps_l2[:], lhsT=t_urp[:], rhs=t_fcwb[:])
                t_nm2 = wpool.tile([BC, 1], F32, tag="fin")
                nc.vector.tensor_reduce(out=t_nm2[:], in_=ps_l2[:],
                                        axis=mybir.AxisListType.X,
                                        op=ALU.max, negate=True)
                t_e2 = wpool.tile([BC, NCLASS], F32, tag="fin2")
                t_s2 = wpool.tile([BC, 1], F32, tag="fin3")
                nc.scalar.activation(out=t_e2[:], in_=ps_l2[:], func=AF.Exp,
                                     bias=t_nm2[:, :1], scale=1.0,
                                     accum_out=t_s2[:, :1])
                t_r2 = wpool.tile([BC, 1], F32, tag="fin4")
                nc.vector.reciprocal(out=t_r2[:], in_=t_s2[:])
                t_o = wpool.tile([BC, NCLASS], F32, tag="fin5")
                nc.vector.tensor_scalar(out=t_o[:], in0=t_e2[:], scalar1=t_r2[:, :1],
                                        scalar2=None, op0=ALU.mult)
                nc.sync.dma_start(out=p_out[:], in_=t_o[:])

            if repeat == 1:
                body()
            else:
                with tc.For_i(0, repeat, 1):
                    body()

    nc.finalize()
    return nc


def prep_in_maps(inputs):
    user = np.asarray(inputs["user"]).astype(np.int32).reshape(B)
    item = np.asarray(inputs["item"]).astype(np.int32).reshape(B)
    neighbor = np.asarray(inputs["neighbor"]).astype(np.int32).reshape(B, S)
    seq = np.asarray(inputs["seq"]).astype(np.int32).reshape(B, S)

    f32 = lambda x: np.ascontiguousarray(np.asarray(x, dtype=np.float32))
    bf16 = lambda x: np.ascontiguousarray(
        np.asarray(x, dtype=np.float32).astype(ml_dtypes.bfloat16))

    uemb = f32(inputs["user_emb_table"])
    item_bf = bf16(inputs["item_emb_table"])
    # both LCU tables in one row: (t, e_msb, r, e_low) so the first fold's
    # halves are contiguous 2048-elem chunks per table
    perm = lambda t: t.reshape(ALLSEQ, R, 2, E // 2).transpose(0, 2, 1, 3).reshape(
        ALLSEQ, R * E)
    lcu_cat = np.ascontiguousarray(np.concatenate(
        [perm(bf16(inputs["item_user_LCU"])),
         perm(bf16(inputs["user_item_LCU"]))], axis=1))
    biases = np.ascontiguousarray(np.stack(
        [f32(inputs["bq"]), f32(inputs["bk"]),
         f32(inputs["bv"]), f32(inputs["bo"])], axis=1))
    sel_eh = np.zeros((E, H), np.float32)
    sel_eh[np.arange(E), np.arange(E) // D] = 1.0
    fc_wb = np.ascontiguousarray(np.concatenate(
        [f32(inputs["fc_w"]), f32(inputs["fc_b"]).reshape(1, NCLASS)], axis=0))

    shared = {
        "uemb": uemb, "item_bf": item_bf, "lcu_cat": lcu_cat,
        "wq": f32(inputs["wq"]), "wk": bf16(inputs["wk"]),
        "wv": bf16(inputs["wv"]), "wo": f32(inputs["wo"]),
        "biases": biases, "sel_eh": np.ascontiguousarray(sel_eh.astype(ml_dtypes.bfloat16)),
        "sel_he": np.ascontiguousarray(sel_eh.T.astype(ml_dtypes.bfloat16)), "fc_wb": fc_wb,
    }
    in_maps = []
    for c in range(NCORES):
        bsl = slice(c * BC, (c + 1) * BC)
        nb = neighbor[bsl].reshape(J)
        sq = seq[bsl].reshape(J)
        itx = np.repeat(item[bsl], S)
        col = lambda x: np.ascontiguousarray(x.reshape(G, 128).T.astype(np.int32))
        in_maps.append({
            **shared,
            "user_i": np.ascontiguousarray(user[bsl].reshape(BC, 1)),
            "nbg_i": col(nb), "seq_i": col(sq), "itg_i": col(itx),
            "negmask": np.ascontiguousarray(
                (-1e9 * (nb <= 0)).astype(ml_dtypes.bfloat16).reshape(1, J)),
        })
    return in_maps


_NC_CACHE = {}


def kernel(**inputs):
    if "nc" not in _NC_CACHE:
        _NC_CACHE["nc"] = build_nc(repeat=1)
    nc = _NC_CACHE["nc"]
    in_maps = prep_in_maps(inputs)
    res = run_bass_kernel_spmd(nc, in_maps, core_ids=list(range(NCORES)))
    return np.concatenate([res.results[c]["out"] for c in range(NCORES)], axis=0)
